# revision 1
# baseline (speedup 1.0000x reference)
"""Trainium2 Bass kernel for nn_AttentionKernelIntegral (linear attention).

Math (per batch b, head h):
    q = x @ Wq^T                      [N, 512]  (no norm)
    k = inorm(x @ Wk^T)               per-(n,h) mean/var over d=64, biased
    v = inorm(x @ Wv^T)
    dots_h = k_h^T v_h                [64, 64]  (contract over ALL N)
    u_h = q_h @ dots_h / N
    out = u @ Wo^T + bo               [N, 256]

Sharding: rows (N) split across 8 cores; only the tiny [B,H,64,64] dots
tensor is all-reduced.  Per-core dataflow (all shapes per batch):
    x [1024,256] --PE transpose--> xT [256,1024]
    k,v = (xT)^T @ W{k,v}T   (fp32r matmuls, moving dim 512)
    bn_stats -> per-(row,head) mean/rstd; normalized k~,v~ cast to fp16,
    with rstd_k*rstd_v folded into v~ (k,v only feed the dots matmul)
    dots: per head-pair packed [128,128] fp16 matmuls accumulated over row
    tiles (diagonal 64x64 blocks extracted afterwards)
    AllReduce(dots) across 8 cores
    qT = WqT^T @ xT (fp16 out), u^T = blockdiag(dots/N) @ qT (fp16)
    out = (uT)^T @ WoT + bo (fp32r), DMA out row-contiguous
"""

import os
import sys

import numpy as np

for _p in ("/opt/trn_rl_repo", os.path.expanduser("~/.axon_site/_ro/trn_rl_repo")):
    if os.path.isdir(_p) and _p not in sys.path:
        sys.path.insert(0, _p)

from contextlib import ExitStack

import concourse.bass as bass
import concourse.mybir as mybir
import concourse.tile as tile
from concourse import bacc
from concourse.bass_utils import run_bass_kernel_spmd
from concourse.masks import make_identity

F32 = mybir.dt.float32
F32R = mybir.dt.float32r
F16 = mybir.dt.float16

B, CIN = 4, 256
H, D = 8, 64
INNER, COUT = 512, 256
EPS = 1e-5
NCORES = 8
N_FULL = 8192
NPAIR = H // 2  # head pairs


def _build(n_chunk, n_full=N_FULL, ncores=NCORES):
    """Build the per-core SPMD Bass program. n_chunk rows per batch per core."""
    NT = n_chunk // 128  # 128-row tiles per batch
    nc = bacc.Bacc(
        "TRN2", target_bir_lowering=False, debug=False, num_devices=ncores)

    x_d = nc.declare_dram_parameter("x", [B, n_chunk, CIN], F32, isOutput=False)
    wq_d = nc.declare_dram_parameter("wq", [INNER, CIN], F32, isOutput=False)
    wk_d = nc.declare_dram_parameter("wk", [INNER, CIN], F32, isOutput=False)
    wv_d = nc.declare_dram_parameter("wv", [INNER, CIN], F32, isOutput=False)
    wo_d = nc.declare_dram_parameter("wo", [COUT, INNER], F32, isOutput=False)
    bo_d = nc.declare_dram_parameter("bo", [1, COUT], F32, isOutput=False)
    out_d = nc.declare_dram_parameter("out", [B, n_chunk, COUT], F32, isOutput=True)

    with ExitStack() as ctx:
        tc = ctx.enter_context(tile.TileContext(nc))
        _body(ctx, tc, nc, NT, n_full, ncores,
              x_d, wq_d, wk_d, wv_d, wo_d, bo_d, out_d)
    nc.compile()
    return nc


def _body(ctx, tc, nc, NT, n_full, ncores,
          x_d, wq_d, wk_d, wv_d, wo_d, bo_d, out_d):
    n_chunk = NT * 128

    # ---------------- pools ----------------
    # PSUM: 8 banks total. xpose(2) + kv(2) + dots(2) + big(2) = 8.
    xpose_ps = ctx.enter_context(tc.tile_pool(name="xpose_ps", bufs=1, space="PSUM"))
    kv_ps = ctx.enter_context(tc.tile_pool(name="kv_ps", bufs=3, space="PSUM"))
    dots_ps = ctx.enter_context(tc.tile_pool(name="dots_ps", bufs=2, space="PSUM"))
    big_ps = ctx.enter_context(tc.tile_pool(name="big_ps", bufs=2, space="PSUM"))

    consts = ctx.enter_context(tc.tile_pool(name="consts", bufs=1))
    wload = ctx.enter_context(tc.tile_pool(name="wload", bufs=2))
    x_pool = ctx.enter_context(tc.tile_pool(name="x_pool", bufs=4))
    xT_pool = ctx.enter_context(tc.tile_pool(name="xT_pool", bufs=2 * B))
    raw_pool = ctx.enter_context(tc.tile_pool(name="raw_pool", bufs=2))
    stats_pool = ctx.enter_context(tc.tile_pool(name="stats_pool", bufs=2))
    small_pool = ctx.enter_context(tc.tile_pool(name="small_pool", bufs=4))
    vtmp_pool = ctx.enter_context(tc.tile_pool(name="vtmp_pool", bufs=2))
    kt_pool = ctx.enter_context(tc.tile_pool(name="kt_pool", bufs=2 * NT))
    vt_pool = ctx.enter_context(tc.tile_pool(name="vt_pool", bufs=2 * NT))
    qT_pool = ctx.enter_context(tc.tile_pool(name="qT_pool", bufs=4 * B))
    uT_pool = ctx.enter_context(tc.tile_pool(name="uT_pool", bufs=8))
    bd_pool = ctx.enter_context(tc.tile_pool(name="bd_pool", bufs=8))
    out_pool = ctx.enter_context(tc.tile_pool(name="out_pool", bufs=6))
    dram = ctx.enter_context(tc.tile_pool(name="dram", bufs=1, space="DRAM"))

    # ---------------- constants / weights ----------------
    ident = consts.tile([128, 128], F16, tag="ident")
    make_identity(nc, ident[:])

    wq_t = [consts.tile([128, INNER], F16, tag=f"wq_t{c}", name=f"wq_t{c}") for c in range(2)]
    wkv_t = [consts.tile([128, 2 * INNER], F16, tag=f"wkv_t{c}", name=f"wkv_t{c}") for c in range(2)]
    wo_t = [consts.tile([128, COUT], F16, tag=f"wo_t{j}", name=f"wo_t{j}") for j in range(4)]

    def load_transposed(w_d, n_rows, store):
        # w_d: [n_rows, CIN] natural; store(ei, cs, psum[128c,128r]) writes dest.
        for ei in range(n_rows // 128):
            wn = wload.tile([128, CIN], F16, tag="wn")
            nc.gpsimd.dma_start(wn[:], w_d[ei * 128:(ei + 1) * 128, :])
            for cs in range(2):
                ps = xpose_ps.tile([128, 128], F16, tag="t")
                nc.tensor.transpose(ps[:], wn[:, cs * 128:(cs + 1) * 128], ident[:])
                store(ei, cs, ps)

    load_transposed(
        wq_d, INNER,
        lambda ei, cs, ps: nc.scalar.copy(wq_t[cs][:, ei * 128:(ei + 1) * 128], ps[:]))
    load_transposed(
        wk_d, INNER,
        lambda ei, cs, ps: nc.scalar.copy(wkv_t[cs][:, ei * 128:(ei + 1) * 128], ps[:]))
    load_transposed(
        wv_d, INNER,
        lambda ei, cs, ps: nc.scalar.copy(
            wkv_t[cs][:, INNER + ei * 128:INNER + (ei + 1) * 128], ps[:]))

    # WoT: Wo [COUT, INNER] -> wo_t[j] [128e, COUT]
    for oi in range(COUT // 128):
        wn = wload.tile([128, INNER], F16, tag="wn2")
        nc.gpsimd.dma_start(wn[:], wo_d[oi * 128:(oi + 1) * 128, :])
        for j in range(4):
            ps = xpose_ps.tile([128, 128], F16, tag="t")
            nc.tensor.transpose(ps[:], wn[:, j * 128:(j + 1) * 128], ident[:])
            nc.scalar.copy(wo_t[j][:, oi * 128:(oi + 1) * 128], ps[:])

    # bias broadcast [128, COUT] via ones outer product
    bo_sb = consts.tile([1, COUT], F32, tag="bo_sb")
    nc.sync.dma_start(bo_sb[:], bo_d[:])
    ones1 = consts.tile([1, 128], F32, tag="ones1")
    nc.gpsimd.memset(ones1[:], 1.0)
    bias_ps = big_ps.tile([128, 512], F32, tag="t")
    nc.tensor.matmul(bias_ps[:, :COUT], ones1[:], bo_sb[:], start=True, stop=True)
    bias_bc = consts.tile([128, COUT], F32, tag="bias_bc")
    nc.scalar.copy(bias_bc[:], bias_ps[:, :COUT])

    # per-head mean weights: msum_t[cs][c, 16] = sum_d wkv_t[cs][c, (kv,h,d)]
    m16_t = []
    for cs in range(2):
        msf = wload.tile([128, 16], F32, tag="msf", name=f"msf{cs}")
        nc.vector.reduce_sum(msf[:], wkv_t[cs][:].rearrange(
            "p (g d) -> p g d", d=D), axis=mybir.AxisListType.X)
        m16 = consts.tile([128, 16], F16, tag=f"m16_{cs}", name=f"m16_{cs}")
        nc.scalar.copy(m16[:], msf[:])
        m16_t.append(m16)

    # dots staging: [128, B * NPAIR * 64]
    dcols = B * NPAIR * 64
    dots_l = consts.tile([128, dcols], F32, tag="dots_l")
    dots_a = consts.tile([128, dcols], F32, tag="dots_a")

    xT_all = {}   # (b, cs) -> [128, n_chunk] f32
    kt_all = {}   # (b, nt) -> [128, 512] f16 (k - mean)
    vt_all = {}   # (b, nt) -> [128, 512] f16 ((v - mean) * rstd_k * rstd_v)

    # ---------------- phase 1: per-batch projections, norm, dots ----------------
    for b in range(B):
        # x load + transpose
        for cs in range(2):
            xT_all[(b, cs)] = xT_pool.tile([128, n_chunk], F16, tag="xT", name=f"xT_{b}_{cs}")
        for nt in range(NT):
            x_t = x_pool.tile([128, CIN], F16, tag="x")
            nc.gpsimd.dma_start(x_t[:], x_d[b, nt * 128:(nt + 1) * 128, :])
            for cs in range(2):
                ps = xpose_ps.tile([128, 128], F16, tag="t")
                nc.tensor.transpose(ps[:], x_t[:, cs * 128:(cs + 1) * 128], ident[:])
                nc.scalar.copy(xT_all[(b, cs)][:, nt * 128:(nt + 1) * 128], ps[:])

        # k,v projections + per-(row,head) sum / sum-of-squares + copy to sbuf
        kraw = raw_pool.tile([128, NT * 512], F16, tag="kraw")
        vraw = raw_pool.tile([128, NT * 512], F16, tag="vraw")
        ksum = stats_pool.tile([128, NT * 8], F32, tag="ksum")
        vsum = stats_pool.tile([128, NT * 8], F32, tag="vsum")
        ksq = stats_pool.tile([128, NT * 8], F32, tag="ksq")
        vsq = stats_pool.tile([128, NT * 8], F32, tag="vsq")
        ksumv = ksum.rearrange("p (t h) -> p t h", h=8)
        vsumv = vsum.rearrange("p (t h) -> p t h", h=8)
        ksqv = ksq.rearrange("p (t h) -> p t h", h=8)
        vsqv = vsq.rearrange("p (t h) -> p t h", h=8)
        for nt in range(NT):
            kps = kv_ps.tile([128, 512], F32, tag="t")
            vps = kv_ps.tile([128, 512], F32, tag="t")
            for cs in range(2):
                xT_sl = xT_all[(b, cs)][:, nt * 128:(nt + 1) * 128]
                nc.tensor.matmul(kps[:], xT_sl, wkv_t[cs][:, :INNER],
                                 start=(cs == 0), stop=(cs == 1))
                nc.tensor.matmul(vps[:], xT_sl, wkv_t[cs][:, INNER:],
                                 start=(cs == 0), stop=(cs == 1))
            mps = dots_ps.tile([128, 16], F32, tag="t", name="mps")
            for cs in range(2):
                xT_sl = xT_all[(b, cs)][:, nt * 128:(nt + 1) * 128]
                nc.tensor.matmul(mps[:], xT_sl, m16_t[cs][:],
                                 start=(cs == 0), stop=(cs == 1))
            nc.vector.tensor_copy(ksumv[:, nt, :], mps[:, 0:8])
            nc.vector.tensor_copy(vsumv[:, nt, :], mps[:, 8:16])
            kr_sl = kraw[:, nt * 512:(nt + 1) * 512]
            vr_sl = vraw[:, nt * 512:(nt + 1) * 512]
            nc.scalar.copy(kr_sl, kps[:])
            nc.scalar.copy(vr_sl, vps[:])
            sqk = vtmp_pool.tile([128, 512], F16, tag="sq", bufs=3)
            sqv = vtmp_pool.tile([128, 512], F16, tag="sq", bufs=3)
            nc.vector.tensor_tensor(sqk[:], kr_sl, kr_sl, op=mybir.AluOpType.mult)
            nc.vector.tensor_tensor(sqv[:], vr_sl, vr_sl, op=mybir.AluOpType.mult)
            nc.vector.reduce_sum(ksqv[:, nt, :],
                                 sqk.rearrange("p (h d) -> p h d", d=D),
                                 axis=mybir.AxisListType.X)
            nc.vector.reduce_sum(vsqv[:, nt, :],
                                 sqv.rearrange("p (h d) -> p h d", d=D),
                                 axis=mybir.AxisListType.X)

        # stats -> mean, rstd  (all [128, NT*8])
        def combine(sums, sumsq, tagp):
            mean = small_pool.tile([128, NT * 8], F32, tag=f"mean{tagp}")
            msq = small_pool.tile([128, NT * 8], F32, tag=f"msq{tagp}")
            var = small_pool.tile([128, NT * 8], F32, tag=f"var{tagp}")
            rstd = small_pool.tile([128, NT * 8], F32, tag=f"rstd{tagp}")
            nc.vector.tensor_scalar_mul(mean[:], sums[:], 1.0 / D)
            nc.vector.tensor_tensor(msq[:], mean[:], mean[:], op=mybir.AluOpType.mult)
            nc.vector.scalar_tensor_tensor(
                var[:], sumsq[:], 1.0 / D, msq[:],
                op0=mybir.AluOpType.mult, op1=mybir.AluOpType.subtract)
            nc.vector.tensor_scalar_add(var[:], var[:], EPS)
            nc.scalar.activation(var[:], var[:], mybir.ActivationFunctionType.Sqrt)
            nc.vector.reciprocal(rstd[:], var[:])
            return mean, rstd

        kmean, krstd = combine(ksum, ksq, "k")
        vmean, vrstd = combine(vsum, vsq, "v")
        w_sc = small_pool.tile([128, NT * 8], F32, tag="wsc")
        nc.vector.tensor_tensor(w_sc[:], krstd[:], vrstd[:], op=mybir.AluOpType.mult)

        kmv = kmean.rearrange("p (t h) -> p t h", h=8)
        vmv = vmean.rearrange("p (t h) -> p t h", h=8)
        wv_ = w_sc.rearrange("p (t h) -> p t h", h=8)
        krv = kraw.rearrange("p (t h d) -> p t h d", h=8, d=D)
        vrv = vraw.rearrange("p (t h d) -> p t h d", h=8, d=D)

        # apply: kt = k - mean (f16); vt = (v - mean) * w (f16)
        for nt in range(NT):
            kt = kt_pool.tile([128, 512], F16, tag="kt")
            vt = vt_pool.tile([128, 512], F16, tag="vt")
            kt_all[(b, nt)] = kt
            vt_all[(b, nt)] = vt
            ktv = kt.rearrange("p (h d) -> p h d", d=D)
            vtv = vt.rearrange("p (h d) -> p h d", d=D)
            nc.vector.tensor_tensor(
                ktv, krv[:, nt, :, :],
                kmv[:, nt, :].broadcast_to([128, 8, D]),
                op=mybir.AluOpType.subtract)
            vtmp = vtmp_pool.tile([128, 512], F16, tag="vtmp")
            vtmpv = vtmp.rearrange("p (h d) -> p h d", d=D)
            nc.vector.tensor_tensor(
                vtmpv, vrv[:, nt, :, :],
                vmv[:, nt, :].broadcast_to([128, 8, D]),
                op=mybir.AluOpType.subtract)
            nc.vector.tensor_tensor(
                vtv, vtmpv,
                wv_[:, nt, :].broadcast_to([128, 8, D]),
                op=mybir.AluOpType.mult)

        # dots: per head pair p, accumulate k_pair^T v_pair over row tiles
        for p in range(NPAIR):
            acc = dots_ps.tile([128, 128], F32, tag="t")
            for nt in range(NT):
                nc.tensor.matmul(
                    acc[:],
                    kt_all[(b, nt)][:, p * 128:(p + 1) * 128],
                    vt_all[(b, nt)][:, p * 128:(p + 1) * 128],
                    start=(nt == 0), stop=(nt == NT - 1))
            col = (b * NPAIR + p) * 64
            nc.vector.tensor_copy(dots_l[0:64, col:col + 64], acc[0:64, 0:64])
            nc.vector.tensor_copy(dots_l[64:128, col:col + 64], acc[64:128, 64:128])

        bcols = NPAIR * 64
        cc_in = dram.tile([128, bcols], F32, tag="cc_in", bufs=B, name=f"cc_in{b}")
        cc_out = dram.tile([128, bcols], F32, tag="cc_out", bufs=B, name=f"cc_out{b}")
        bsl = slice(b * bcols, (b + 1) * bcols)
        nc.sync.dma_start(cc_in[:], dots_l[:, bsl])
        nc.gpsimd.collective_compute(
            "AllReduce", mybir.AluOpType.add,
            replica_groups=[list(range(ncores))],
            ins=[cc_in.opt()], outs=[cc_out.opt()])
        nc.sync.dma_start(dots_a[:, bsl], cc_out[:])


    # ---------------- phase 3: q projection (overlaps all-reduce) ----------------
    qT_all = {}
    for b in range(B):
        for j in range(4):
            qT = qT_pool.tile([128, n_chunk], F16, tag="qT")
            qT_all[(b, j)] = qT
            for ch in range(n_chunk // 512):
                qps = dots_ps.tile([128, 512], F32, tag="t", name="qps")
                for cs in range(2):
                    nc.tensor.matmul(
                        qps[:],
                        wq_t[cs][:, j * 128:(j + 1) * 128],
                        xT_all[(b, cs)][:, ch * 512:(ch + 1) * 512],
                        start=(cs == 0), stop=(cs == 1))
                nc.scalar.copy(qT[:, ch * 512:(ch + 1) * 512], qps[:])

    # ---------------- phase 4: u = blockdiag(dots/N) @ qT, out projection ----------------
    for b in range(B):
        uT_b = []
        for j in range(4):
            bd = bd_pool.tile([128, 128], F16, tag="bd")
            nc.gpsimd.memset(bd[:], 0.0)
            col = (b * NPAIR + j) * 64
            nc.scalar.activation(bd[0:64, 0:64], dots_a[0:64, col:col + 64],
                                 mybir.ActivationFunctionType.Copy, scale=1.0 / n_full)
            nc.scalar.activation(bd[64:128, 64:128], dots_a[64:128, col:col + 64],
                                 mybir.ActivationFunctionType.Copy, scale=1.0 / n_full)
            uT = uT_pool.tile([128, n_chunk], F16, tag="uT")
            uT_b.append(uT)
            for ch in range(n_chunk // 512):
                ups = kv_ps.tile([128, 512], F32, tag="t", name="ups")
                nc.tensor.matmul(ups[:], bd[:],
                                 qT_all[(b, j)][:, ch * 512:(ch + 1) * 512],
                                 start=True, stop=True)
                nc.scalar.copy(uT[:, ch * 512:(ch + 1) * 512], ups[:])

        for nt in range(NT):
            ops = big_ps.tile([128, 512], F32, tag="t")
            for j in range(4):
                nc.tensor.matmul(
                    ops[:, :COUT],
                    uT_b[j][:, nt * 128:(nt + 1) * 128],
                    wo_t[j][:],
                    start=(j == 0), stop=(j == 3))
            osb = out_pool.tile([128, COUT], F32, tag="osb")
            nc.vector.tensor_tensor(osb[:], ops[:, :COUT], bias_bc[:],
                                    op=mybir.AluOpType.add)
            nc.sync.dma_start(out_d[b, nt * 128:(nt + 1) * 128, :], osb[:])


_NC_CACHE = {}


def _get_nc(n_chunk, n_full, ncores):
    key = (n_chunk, n_full, ncores)
    if key not in _NC_CACHE:
        _NC_CACHE[key] = _build(n_chunk, n_full, ncores)
    return _NC_CACHE[key]


def _make_in_maps(u_x, Wq, Wk, Wv, Wo, bo, ncores):
    n = u_x.shape[1]
    n_chunk = n // ncores
    wq = np.ascontiguousarray(np.asarray(Wq, np.float32))
    wk = np.ascontiguousarray(np.asarray(Wk, np.float32))
    wv = np.ascontiguousarray(np.asarray(Wv, np.float32))
    wo = np.ascontiguousarray(np.asarray(Wo, np.float32))
    bo2 = np.ascontiguousarray(np.asarray(bo, np.float32).reshape(1, -1))
    u_x = np.asarray(u_x, np.float32)
    maps = []
    for c in range(ncores):
        maps.append({
            "x": np.ascontiguousarray(u_x[:, c * n_chunk:(c + 1) * n_chunk, :]),
            "wq": wq, "wk": wk, "wv": wv, "wo": wo, "bo": bo2,
        })
    return maps, n_chunk


def _install_ntff_hook():
    """Provide antenv.axon_hooks (missing in this image) so trace=True works."""
    import types
    try:
        from antenv.axon_hooks import get_axon_ntff_profile_hook  # noqa: F401
        return  # real module present
    except ImportError:
        pass
    try:
        import antenv
        mod = types.ModuleType("antenv.axon_hooks")
        _state = {"hook": None}
        mod.set_axon_ntff_profile_hook = lambda h: _state.__setitem__("hook", h)
        mod.get_axon_ntff_profile_hook = lambda: _state["hook"]
        sys.modules["antenv.axon_hooks"] = mod
        antenv.axon_hooks = mod
        boot_dir = "/root/.axon_site/trn_agent_boot"
        if boot_dir not in sys.path and os.path.isdir(boot_dir):
            sys.path.insert(0, boot_dir)
        import trn_boot
        so_path = "/opt/axon/libaxon_pjrt.so"
        if os.path.exists(so_path):
            hook = trn_boot._ntff_profile_via_ctypes(so_path)
            if hook is not None:
                mod.set_axon_ntff_profile_hook(hook)
    except Exception as e:  # tracing is best-effort; never break the run path
        print(f"ntff hook install failed: {e}", file=sys.stderr)


def run(u_x, Wq, Wk, Wv, Wo, bo, n_full=None, ncores=NCORES, trace=False,
        tmpdir=None):
    if trace:
        _install_ntff_hook()
    n = u_x.shape[1]
    if n_full is None:
        n_full = n
    in_maps, n_chunk = _make_in_maps(u_x, Wq, Wk, Wv, Wo, bo, ncores)
    nc = _get_nc(n_chunk, n_full, ncores)
    res = run_bass_kernel_spmd(nc, in_maps, list(range(ncores)), trace=trace,
                               tmpdir=tmpdir)
    outs = [np.asarray(res.results[c]["out"]) for c in range(ncores)]
    full = np.concatenate(outs, axis=1).astype(np.float32)
    return full, res


def kernel(u_x, pos_x=None, Wq=None, Wk=None, Wv=None, Wo=None, bo=None):
    full, _ = run(np.asarray(u_x, np.float32), Wq, Wk, Wv, Wo, bo)
    return full



# revision 7
# speedup vs baseline: 1.0712x; 1.0712x over previous
"""Trainium2 Bass kernel for nn_AttentionKernelIntegral (linear attention).

Math (per batch b, head h):
    q = x @ Wq^T                      [N, 512]  (no norm)
    k = inorm(x @ Wk^T)               per-(n,h) mean/var over d=64, biased
    v = inorm(x @ Wv^T)
    dots_h = k_h^T v_h                [64, 64]  (contract over ALL N)
    u_h = q_h @ dots_h / N
    out = u @ Wo^T + bo               [N, 256]

Key algebraic restructure vs the straightforward version:
  - Mean-centering over d is right-multiplication by C = I - J/64, so
        dots_h = C (K_h^T diag(a) V_h) C,   a_n = rsqrt((var_k+eps)(var_v+eps))
    i.e. only the per-row scale a touches the big [N,512] tensors (folded
    into k only); centering moves to tiny [64,64] matmuls AFTER the
    AllReduce of the un-centered M = K^T diag(a) V.
  - u = Q dots and out = u Wo^T fold into per-batch weights:
        G_b = Wo blockdiag(dots_b / N) Wq ;  out = X G_b^T + bo
    eliminating the q projection staging and the u apply entirely.

Sharding: rows (N) split across 8 cores; only [B,H,64,64] M is all-reduced.
Per-core dataflow per batch (n_chunk rows):
    x --gpsimd cast DMA--> f16 --PE transpose--> xT [256, n_chunk]
    k,v = xT^T @ W{k,v}^T (f16 PE, PSUM) --scalar/gpsimd--> f16 kraw/vraw
    bn_stats(kraw/vraw) -> var -> a;  kt = kraw * a (vector)
    M(pair p) += kt^T vraw  (PE, accumulated over row tiles)
    AllReduce(M diag blocks);  then per pair:
      R = M_z^T (C' Wq),  Wt = C' R,  (C' = (I-J/64)/sqrt(N))
      G^T = sum_j Wt_j^T Wo^T_j,  out = X G^T + bo
"""

import os
import sys

import numpy as np

for _p in ("/opt/trn_rl_repo", os.path.expanduser("~/.axon_site/_ro/trn_rl_repo")):
    if os.path.isdir(_p) and _p not in sys.path:
        sys.path.insert(0, _p)

from contextlib import ExitStack

import concourse.bass as bass
import concourse.mybir as mybir
import concourse.tile as tile
from concourse import bacc
from concourse.bass_utils import run_bass_kernel_spmd
from concourse.masks import make_identity

F32 = mybir.dt.float32
F16 = mybir.dt.float16

B, CIN = 4, 256
H, D = 8, 64
INNER, COUT = 512, 256
EPS = 1e-5
NCORES = 8
N_FULL = 8192
NPAIR = H // 2  # head pairs


def _build(n_chunk, n_full=N_FULL, ncores=NCORES):
    """Build the per-core SPMD Bass program. n_chunk rows per batch per core."""
    NT = n_chunk // 128  # 128-row tiles per batch
    nc = bacc.Bacc(
        "TRN2", target_bir_lowering=False, debug=False, num_devices=ncores)

    x_d = nc.declare_dram_parameter("x", [B, n_chunk, CIN], F32, isOutput=False)
    wq_d = nc.declare_dram_parameter("wq", [INNER, CIN], F32, isOutput=False)
    wk_d = nc.declare_dram_parameter("wk", [INNER, CIN], F32, isOutput=False)
    wv_d = nc.declare_dram_parameter("wv", [INNER, CIN], F32, isOutput=False)
    wo_d = nc.declare_dram_parameter("wo", [COUT, INNER], F32, isOutput=False)
    bo_d = nc.declare_dram_parameter("bo", [1, COUT], F32, isOutput=False)
    out_d = nc.declare_dram_parameter("out", [B, n_chunk, COUT], F32, isOutput=True)

    with ExitStack() as ctx:
        tc = ctx.enter_context(tile.TileContext(nc))
        _body(ctx, tc, nc, NT, n_full, ncores,
              x_d, wq_d, wk_d, wv_d, wo_d, bo_d, out_d)
    nc.compile()
    return nc


def _bn_stats_seg(nc, out, in_):
    """bn_stats with the segmented (3D) access pattern preserved.

    nc.vector.bn_stats lowers with opt=True which merges contiguous free
    dims, collapsing [p, h, d] to [p, h*d] and producing a single segment.
    Lower with opt=False so each h gets its own 6-tuple of stats.
    """
    eng = nc.vector
    return eng.add_instruction(
        mybir.InstBNStats(
            name=nc.get_next_instruction_name(),
            ins=[eng.lower_ap(in_, opt=False)],
            outs=[eng.lower_ap(out, opt=False)],
        )
    )


def _body(ctx, tc, nc, NT, n_full, ncores,
          x_d, wq_d, wk_d, wv_d, wo_d, bo_d, out_d):
    n_chunk = NT * 128

    # ---------------- pools ----------------
    # PSUM budget (8 banks): xp(2) + kv(3) + dots(1) + sm(2) = 8
    xpose_ps = ctx.enter_context(tc.tile_pool(name="xpose_ps", bufs=2, space="PSUM"))
    kv_ps = ctx.enter_context(tc.tile_pool(name="kv_ps", bufs=3, space="PSUM"))
    dots_ps = ctx.enter_context(tc.tile_pool(name="dots_ps", bufs=1, space="PSUM"))
    sm_ps = ctx.enter_context(tc.tile_pool(name="sm_ps", bufs=2, space="PSUM"))

    consts = ctx.enter_context(tc.tile_pool(name="consts", bufs=1))
    wload = ctx.enter_context(tc.tile_pool(name="wload", bufs=2))
    x_pool = ctx.enter_context(tc.tile_pool(name="x_pool", bufs=2 * NT))
    xT_pool = ctx.enter_context(tc.tile_pool(name="xT_pool", bufs=1))
    kv_sb = ctx.enter_context(tc.tile_pool(name="kv_sb", bufs=2 * NT))
    stats_pool = ctx.enter_context(tc.tile_pool(name="stats_pool", bufs=2))
    small_pool = ctx.enter_context(tc.tile_pool(name="small_pool", bufs=2))
    p2_pool = ctx.enter_context(tc.tile_pool(name="p2_pool", bufs=4))
    out_pool = ctx.enter_context(tc.tile_pool(name="out_pool", bufs=4))
    dram = ctx.enter_context(tc.tile_pool(name="dram", bufs=1, space="DRAM"))

    # ---------------- constants / weights ----------------
    ident = consts.tile([128, 128], F16, tag="ident")
    make_identity(nc, ident[:])

    # C' = (I - J/64)/sqrt(N), blockdiag over the head pair: zero cross blocks
    rtn = float(np.sqrt(float(n_full)))
    cmat = consts.tile([128, 128], F16, tag="cmat")
    nc.gpsimd.memset(cmat[:], 0.0)
    nc.gpsimd.memset(cmat[0:64, 0:64], -1.0 / (64.0 * rtn))
    nc.gpsimd.memset(cmat[64:128, 64:128], -1.0 / (64.0 * rtn))
    nc.vector.scalar_tensor_tensor(
        cmat[:], ident[:], 1.0 / rtn, cmat[:],
        op0=mybir.AluOpType.mult, op1=mybir.AluOpType.add)

    identf = consts.tile([128, 128], F32, tag="identf")
    make_identity(nc, identf[:])

    # transposed W{k,v}: wkv_t[cs][c, ei*128+r] = W{k,v}[ei*128+r, cs*128+c]
    wkv_t = [consts.tile([128, 2 * INNER], F16, tag=f"wkv_t{c}", name=f"wkv_t{c}")
             for c in range(2)]
    for wi, w_d in enumerate((wk_d, wv_d)):
        for ei in range(INNER // 128):
            wn = wload.tile([128, CIN], F16, tag="wn")
            nc.gpsimd.dma_start(wn[:], w_d[ei * 128:(ei + 1) * 128, :])
            for cs in range(2):
                ps = xpose_ps.tile([128, 256], F16, tag="xp", name="ps")
                nc.tensor.transpose(ps[:, 0:128], wn[:, cs * 128:(cs + 1) * 128],
                                    ident[:])
                nc.scalar.copy(
                    wkv_t[cs][:, wi * INNER + ei * 128:wi * INNER + (ei + 1) * 128],
                    ps[:, 0:128])

    # Wo^T: wo_t[j][e, oi*128+o] = Wo[oi*128+o, j*128+e]
    wo_t = [consts.tile([128, COUT], F16, tag=f"wo_t{j}", name=f"wo_t{j}")
            for j in range(4)]
    for oi in range(COUT // 128):
        won = wload.tile([128, INNER], F16, tag="wn2")
        nc.gpsimd.dma_start(won[:], wo_d[oi * 128:(oi + 1) * 128, :])
        for j in range(4):
            ps = xpose_ps.tile([128, 256], F16, tag="xp", name="ps")
            nc.tensor.transpose(ps[:, 0:128], won[:, j * 128:(j + 1) * 128],
                                ident[:])
            nc.scalar.copy(wo_t[j][:, oi * 128:(oi + 1) * 128], ps[:, 0:128])

    # per-head sum weights: m16_t[cs][c, g] = sum_d wkv_t[cs][c, (g d)]
    m16_t = []
    for cs in range(2):
        msf = wload.tile([128, 16], F32, tag="msf", name=f"msf{cs}")
        nc.vector.reduce_sum(msf[:], wkv_t[cs][:].rearrange(
            "p (g d) -> p g d", d=D), axis=mybir.AxisListType.X)
        m16 = consts.tile([128, 16], F16, tag=f"m16_{cs}", name=f"m16_{cs}")
        nc.scalar.copy(m16[:], msf[:])
        m16_t.append(m16)

    # Wq natural rows (per pair) and Wq' = C' @ Wq_pair
    wq_nat = []
    wq_p = []
    for p in range(NPAIR):
        wqn = consts.tile([128, CIN], F16, tag=f"wqn{p}", name=f"wqn{p}")
        nc.gpsimd.dma_start(wqn[:], wq_d[p * 128:(p + 1) * 128, :])
        wq_nat.append(wqn)
    for p in range(NPAIR):
        ps = sm_ps.tile([128, 256], F32, tag="sm", name="ps")
        nc.tensor.matmul(ps[:], cmat[:], wq_nat[p][:], start=True, stop=True)
        wqp = consts.tile([128, CIN], F16, tag=f"wqp{p}", name=f"wqp{p}")
        nc.scalar.copy(wqp[:], ps[:])
        wq_p.append(wqp)

    # bias broadcast [128, COUT] via ones outer product
    bo_sb = consts.tile([1, COUT], F32, tag="bo_sb")
    nc.sync.dma_start(bo_sb[:], bo_d[:])
    ones1 = consts.tile([1, 128], F32, tag="ones1")
    nc.gpsimd.memset(ones1[:], 1.0)
    bias_ps = sm_ps.tile([128, 256], F32, tag="sm", name="bias_ps")
    nc.tensor.matmul(bias_ps[:], ones1[:], bo_sb[:], start=True, stop=True)
    bias_bc = consts.tile([128, COUT], F32, tag="bias_bc")
    nc.scalar.copy(bias_bc[:], bias_ps[:])

    # M_z staging tiles (pre-zeroed; only diag blocks ever written)
    mz_all = {}
    for b in range(B):
        for p in range(NPAIR):
            mz = consts.tile([128, 128], F16, tag=f"mz{b}_{p}", name=f"mz{b}_{p}")
            nc.gpsimd.memset(mz[:], 0.0)
            mz_all[(b, p)] = mz

    # dots staging: [128, B * NPAIR * 64]
    dcols = B * NPAIR * 64
    dots_l = consts.tile([128, dcols], F32, tag="dots_l")
    dots_a = consts.tile([128, dcols], F32, tag="dots_a")

    xT_all = {}   # (b, cs) -> [128, n_chunk] f16
    kraw_all = {}  # (b, nt) -> [128, 1024] f16 (k | v)
    kt_all = {}
    a_all = {}

    # ---------------- phase 1a: x load/transpose, kv proj, stats ----------------
    def phase1a(b):
        x_ts = []
        for nt in range(NT):
            x_t = x_pool.tile([128, CIN], F32, tag="x", name="x_t")
            nc.sync.dma_start(x_t[:], x_d[b, nt * 128:(nt + 1) * 128, :])
            x_ts.append(x_t)
        for cs in range(2):
            xT = xT_pool.tile([128, n_chunk], F16, tag=f"xT{b}_{cs}",
                              name=f"xT{b}_{cs}")
            xT_all[(b, cs)] = xT
            for ntp in range(NT // 2):
                ps = xpose_ps.tile([128, 256], F32, tag="xp", name="ps")
                nc.tensor.transpose(
                    ps[:, 0:128], x_ts[2 * ntp][:, cs * 128:(cs + 1) * 128],
                    identf[:])
                nc.tensor.transpose(
                    ps[:, 128:256], x_ts[2 * ntp + 1][:, cs * 128:(cs + 1) * 128],
                    identf[:])
                nc.scalar.copy(xT[:, ntp * 256:(ntp + 1) * 256], ps[:])

        # S1 = sum_d k|v (from the PE mean-matmul), S2 = sum_d (k|v)^2
        s1 = stats_pool.tile([128, NT, 16], F32, tag="s1", name="s1")
        s2 = stats_pool.tile([128, NT, 16], F32, tag="s2", name="s2")
        for nt in range(NT):
            kps = kv_ps.tile([128, 512], F32, tag="kv", name="kps")
            vps = kv_ps.tile([128, 512], F32, tag="kv", name="vps")
            for cs in range(2):
                xT_sl = xT_all[(b, cs)][:, nt * 128:(nt + 1) * 128]
                nc.tensor.matmul(kps[:], xT_sl, wkv_t[cs][:, :INNER],
                                 start=(cs == 0), stop=(cs == 1))
                nc.tensor.matmul(vps[:], xT_sl, wkv_t[cs][:, INNER:],
                                 start=(cs == 0), stop=(cs == 1))
            mps = sm_ps.tile([128, 256], F32, tag="sm", name="mps")
            for cs in range(2):
                xT_sl = xT_all[(b, cs)][:, nt * 128:(nt + 1) * 128]
                nc.tensor.matmul(mps[:, 0:16], xT_sl, m16_t[cs][:],
                                 start=(cs == 0), stop=(cs == 1))
            kvraw = kv_sb.tile([128, 1024], F16, tag="kvraw", name="kvraw")
            kraw_all[(b, nt)] = kvraw
            nc.scalar.copy(kvraw[:, 0:512], kps[:])
            nc.scalar.copy(kvraw[:, 512:1024], vps[:])
            nc.vector.tensor_copy(s1[:, nt, :], mps[:, 0:16])
            sq = kv_sb.tile([128, 1024], F16, tag="sq", bufs=3, name="sq")
            nc.gpsimd.tensor_tensor(sq[:], kvraw[:], kvraw[:],
                                    op=mybir.AluOpType.mult)
            nc.vector.reduce_sum(s2[:, nt, :],
                                 sq.rearrange("p (g d) -> p g d", d=D),
                                 axis=mybir.AxisListType.X)

        # var*4096 = 64*S2 - S1^2 ;  a = 4096*rsqrt(prodk*prodv)
        t1 = small_pool.tile([128, NT, 16], F32, tag="t1")
        nc.vector.tensor_tensor(t1[:], s1[:], s1[:], op=mybir.AluOpType.mult)
        t2 = small_pool.tile([128, NT, 16], F32, tag="t2")
        nc.vector.scalar_tensor_tensor(
            t2[:], s2[:], 64.0, t1[:],
            op0=mybir.AluOpType.mult, op1=mybir.AluOpType.subtract)
        nc.vector.tensor_scalar_add(t2[:], t2[:], 4096.0 * EPS)
        prod = small_pool.tile([128, NT, 8], F32, tag="prod")
        nc.vector.tensor_tensor(prod[:], t2[:, :, 0:8], t2[:, :, 8:16],
                                op=mybir.AluOpType.mult)
        nc.scalar.activation(prod[:], prod[:], mybir.ActivationFunctionType.Sqrt)
        recp = small_pool.tile([128, NT, 8], F32, tag="recp")
        nc.vector.reciprocal(recp[:], prod[:])
        a_sc = small_pool.tile([128, NT, 8], F16, tag="asc")
        nc.vector.tensor_scalar_mul(a_sc[:], recp[:], 4096.0)
        a_all[b] = a_sc

        for nt in range(NT):
            kt = kv_sb.tile([128, 512], F16, tag="kt", name="kt")
            kt_all[(b, nt)] = kt
            nc.vector.tensor_tensor(
                kt.rearrange("p (h d) -> p h d", d=D),
                kraw_all[(b, nt)][:, 0:512].rearrange("p (h d) -> p h d", d=D),
                a_sc[:, nt, :].broadcast_to([128, 8, D]),
                op=mybir.AluOpType.mult)

    # ---------------- dots + AllReduce ----------------
    def dots_ar(b):
        acc = dots_ps.tile([128, 512], F32, tag="dots", name="acc")
        for p in range(NPAIR):
            for nt in range(NT):
                nc.tensor.matmul(
                    acc[:, p * 128:(p + 1) * 128],
                    kt_all[(b, nt)][:, p * 128:(p + 1) * 128],
                    kraw_all[(b, nt)][:, 512 + p * 128:512 + (p + 1) * 128],
                    start=(nt == 0), stop=(nt == NT - 1))
        for p in range(NPAIR):
            col = (b * NPAIR + p) * 64
            nc.vector.tensor_copy(dots_l[0:64, col:col + 64],
                                  acc[0:64, p * 128:p * 128 + 64])
            nc.vector.tensor_copy(dots_l[64:128, col:col + 64],
                                  acc[64:128, p * 128 + 64:(p + 1) * 128])
        bcols = NPAIR * 64
        cc_in = dram.tile([128, bcols], F32, tag="cc_in", bufs=B, name=f"cc_in{b}")
        cc_out = dram.tile([128, bcols], F32, tag="cc_out", bufs=B, name=f"cc_out{b}")
        bsl = slice(b * bcols, (b + 1) * bcols)
        nc.sync.dma_start(cc_in[:], dots_l[:, bsl])
        nc.gpsimd.collective_compute(
            "AllReduce", mybir.AluOpType.add,
            replica_groups=[list(range(ncores))],
            ins=[cc_in.opt()], outs=[cc_out.opt()])
        nc.sync.dma_start(dots_a[:, bsl], cc_out[:])

    # ---------------- phase 2: fold dots into weights, out proj ----------------
    def phase2(b):
        wt_sb = []
        for p in range(NPAIR):
            col = (b * NPAIR + p) * 64
            mz = mz_all[(b, p)]
            nc.scalar.copy(mz[0:64, 0:64], dots_a[0:64, col:col + 64])
            nc.scalar.copy(mz[64:128, 64:128], dots_a[64:128, col:col + 64])
            ps = sm_ps.tile([128, 256], F32, tag="sm", name="ps_r")
            nc.tensor.matmul(ps[:], mz[:], wq_p[p][:], start=True, stop=True)
            r_sb = p2_pool.tile([128, 256], F16, tag="r_sb", name="r_sb")
            nc.scalar.copy(r_sb[:], ps[:])
            ps2 = sm_ps.tile([128, 256], F32, tag="sm", name="ps_w")
            nc.tensor.matmul(ps2[:], cmat[:], r_sb[:], start=True, stop=True)
            wt = p2_pool.tile([128, 256], F16, tag="wt", bufs=2 * NPAIR, name="wt")
            nc.scalar.copy(wt[:], ps2[:])
            wt_sb.append(wt)
        gt_sb = []
        for cs in range(2):
            psg = sm_ps.tile([128, 256], F32, tag="sm", name="ps_g")
            for j in range(NPAIR):
                nc.tensor.matmul(psg[:], wt_sb[j][:, cs * 128:(cs + 1) * 128],
                                 wo_t[j][:], start=(j == 0), stop=(j == NPAIR - 1))
            gt = p2_pool.tile([128, 256], F16, tag="gt", name="gt")
            nc.scalar.copy(gt[:], psg[:])
            gt_sb.append(gt)
        for nt in range(NT):
            pso = sm_ps.tile([128, 256], F32, tag="sm", name="ps_o")
            for cs in range(2):
                nc.tensor.matmul(pso[:],
                                 xT_all[(b, cs)][:, nt * 128:(nt + 1) * 128],
                                 gt_sb[cs][:], start=(cs == 0), stop=(cs == 1))
            osb = out_pool.tile([128, COUT], F32, tag="osb", name="osb")
            nc.vector.tensor_tensor(osb[:], pso[:], bias_bc[:],
                                    op=mybir.AluOpType.add)
            nc.sync.dma_start(out_d[b, nt * 128:(nt + 1) * 128, :], osb[:])

    # schedule: keep PE fed; dots(b) emitted after phase1a(b+1) so the
    # normalization (vector) of batch b overlaps the projections of b+1;
    # phase2(b) emitted ~2 phases after its AllReduce was issued.
    phase1a(0)
    phase1a(1)
    dots_ar(0)
    phase1a(2)
    dots_ar(1)
    phase2(0)
    phase1a(3)
    dots_ar(2)
    phase2(1)
    dots_ar(3)
    phase2(2)
    phase2(3)


_NC_CACHE = {}


def _get_nc(n_chunk, n_full, ncores):
    key = (n_chunk, n_full, ncores)
    if key not in _NC_CACHE:
        _NC_CACHE[key] = _build(n_chunk, n_full, ncores)
    return _NC_CACHE[key]


def _make_in_maps(u_x, Wq, Wk, Wv, Wo, bo, ncores):
    n = u_x.shape[1]
    n_chunk = n // ncores
    wq = np.ascontiguousarray(np.asarray(Wq, np.float32))
    wk = np.ascontiguousarray(np.asarray(Wk, np.float32))
    wv = np.ascontiguousarray(np.asarray(Wv, np.float32))
    wo = np.ascontiguousarray(np.asarray(Wo, np.float32))
    bo2 = np.ascontiguousarray(np.asarray(bo, np.float32).reshape(1, -1))
    u_x = np.asarray(u_x, np.float32)
    maps = []
    for c in range(ncores):
        maps.append({
            "x": np.ascontiguousarray(u_x[:, c * n_chunk:(c + 1) * n_chunk, :]),
            "wq": wq, "wk": wk, "wv": wv, "wo": wo, "bo": bo2,
        })
    return maps, n_chunk


def _install_ntff_hook():
    """Provide antenv.axon_hooks (missing in this image) so trace=True works."""
    import types
    try:
        from antenv.axon_hooks import get_axon_ntff_profile_hook  # noqa: F401
        return  # real module present
    except ImportError:
        pass
    try:
        import antenv
        mod = types.ModuleType("antenv.axon_hooks")
        _state = {"hook": None}
        mod.set_axon_ntff_profile_hook = lambda h: _state.__setitem__("hook", h)
        mod.get_axon_ntff_profile_hook = lambda: _state["hook"]
        sys.modules["antenv.axon_hooks"] = mod
        antenv.axon_hooks = mod
        boot_dir = "/root/.axon_site/trn_agent_boot"
        if boot_dir not in sys.path and os.path.isdir(boot_dir):
            sys.path.insert(0, boot_dir)
        import trn_boot
        so_path = "/opt/axon/libaxon_pjrt.so"
        if os.path.exists(so_path):
            hook = trn_boot._ntff_profile_via_ctypes(so_path)
            if hook is not None:
                mod.set_axon_ntff_profile_hook(hook)
    except Exception as e:  # tracing is best-effort; never break the run path
        print(f"ntff hook install failed: {e}", file=sys.stderr)


def run(u_x, Wq, Wk, Wv, Wo, bo, n_full=None, ncores=NCORES, trace=False,
        tmpdir=None):
    if trace:
        _install_ntff_hook()
    n = u_x.shape[1]
    if n_full is None:
        n_full = n
    in_maps, n_chunk = _make_in_maps(u_x, Wq, Wk, Wv, Wo, bo, ncores)
    nc = _get_nc(n_chunk, n_full, ncores)
    res = run_bass_kernel_spmd(nc, in_maps, list(range(ncores)), trace=trace,
                               tmpdir=tmpdir)
    outs = [np.asarray(res.results[c]["out"]) for c in range(ncores)]
    full = np.concatenate(outs, axis=1).astype(np.float32)
    return full, res


def kernel(u_x, pos_x=None, Wq=None, Wk=None, Wv=None, Wo=None, bo=None):
    full, _ = run(np.asarray(u_x, np.float32), Wq, Wk, Wv, Wo, bo)
    return full


# revision 8
# speedup vs baseline: 1.1222x; 1.0476x over previous
"""Trainium2 Bass kernel for nn_AttentionKernelIntegral (linear attention).

Math (per batch b, head h):
    q = x @ Wq^T                      [N, 512]  (no norm)
    k = inorm(x @ Wk^T)               per-(n,h) mean/var over d=64, biased
    v = inorm(x @ Wv^T)
    dots_h = k_h^T v_h                [64, 64]  (contract over ALL N)
    u_h = q_h @ dots_h / N
    out = u @ Wo^T + bo               [N, 256]

Key algebraic restructure vs the straightforward version:
  - Mean-centering over d is right-multiplication by C = I - J/64, so
        dots_h = C (K_h^T diag(a) V_h) C,   a_n = rsqrt((var_k+eps)(var_v+eps))
    i.e. only the per-row scale a touches the big [N,512] tensors (folded
    into k only); centering moves to tiny [64,64] matmuls AFTER the
    AllReduce of the un-centered M = K^T diag(a) V.
  - u = Q dots and out = u Wo^T fold into per-batch weights:
        G_b = Wo blockdiag(dots_b / N) Wq ;  out = X G_b^T + bo
    eliminating the q projection staging and the u apply entirely.

Sharding: rows (N) split across 8 cores; only [B,H,64,64] M is all-reduced.
Per-core dataflow per batch (n_chunk rows):
    x --gpsimd cast DMA--> f16 --PE transpose--> xT [256, n_chunk]
    k,v = xT^T @ W{k,v}^T (f16 PE, PSUM) --scalar/gpsimd--> f16 kraw/vraw
    bn_stats(kraw/vraw) -> var -> a;  kt = kraw * a (vector)
    M(pair p) += kt^T vraw  (PE, accumulated over row tiles)
    AllReduce(M diag blocks);  then per pair:
      R = M_z^T (C' Wq),  Wt = C' R,  (C' = (I-J/64)/sqrt(N))
      G^T = sum_j Wt_j^T Wo^T_j,  out = X G^T + bo
"""

import os
import sys

import numpy as np

for _p in ("/opt/trn_rl_repo", os.path.expanduser("~/.axon_site/_ro/trn_rl_repo")):
    if os.path.isdir(_p) and _p not in sys.path:
        sys.path.insert(0, _p)

from contextlib import ExitStack

import concourse.bass as bass
import concourse.mybir as mybir
import concourse.tile as tile
from concourse import bacc
from concourse.bass_utils import run_bass_kernel_spmd
from concourse.masks import make_identity

F32 = mybir.dt.float32
F16 = mybir.dt.float16

B, CIN = 4, 256
H, D = 8, 64
INNER, COUT = 512, 256
EPS = 1e-5
NCORES = 8
N_FULL = 8192
NPAIR = H // 2  # head pairs


def _build(n_chunk, n_full=N_FULL, ncores=NCORES):
    """Build the per-core SPMD Bass program. n_chunk rows per batch per core."""
    NT = n_chunk // 128  # 128-row tiles per batch
    nc = bacc.Bacc(
        "TRN2", target_bir_lowering=False, debug=False, num_devices=ncores)

    x_d = nc.declare_dram_parameter("x", [B, n_chunk, CIN], F32, isOutput=False)
    wq_d = nc.declare_dram_parameter("wq", [INNER, CIN], F32, isOutput=False)
    wk_d = nc.declare_dram_parameter("wk", [INNER, CIN], F32, isOutput=False)
    wv_d = nc.declare_dram_parameter("wv", [INNER, CIN], F32, isOutput=False)
    wo_d = nc.declare_dram_parameter("wo", [COUT, INNER], F32, isOutput=False)
    bo_d = nc.declare_dram_parameter("bo", [1, COUT], F32, isOutput=False)
    out_d = nc.declare_dram_parameter("out", [B, n_chunk, COUT], F32, isOutput=True)

    with ExitStack() as ctx:
        tc = ctx.enter_context(tile.TileContext(nc))
        _body(ctx, tc, nc, NT, n_full, ncores,
              x_d, wq_d, wk_d, wv_d, wo_d, bo_d, out_d)
    nc.compile()
    return nc


def _bn_stats_seg(nc, out, in_):
    """bn_stats with the segmented (3D) access pattern preserved.

    nc.vector.bn_stats lowers with opt=True which merges contiguous free
    dims, collapsing [p, h, d] to [p, h*d] and producing a single segment.
    Lower with opt=False so each h gets its own 6-tuple of stats.
    """
    eng = nc.vector
    return eng.add_instruction(
        mybir.InstBNStats(
            name=nc.get_next_instruction_name(),
            ins=[eng.lower_ap(in_, opt=False)],
            outs=[eng.lower_ap(out, opt=False)],
        )
    )


def _body(ctx, tc, nc, NT, n_full, ncores,
          x_d, wq_d, wk_d, wv_d, wo_d, bo_d, out_d):
    n_chunk = NT * 128

    # ---------------- pools ----------------
    # PSUM budget (8 banks): xp(2) + kv(3) + dots(1) + sm(2) = 8
    xpose_ps = ctx.enter_context(tc.tile_pool(name="xpose_ps", bufs=2, space="PSUM"))
    kv_ps = ctx.enter_context(tc.tile_pool(name="kv_ps", bufs=3, space="PSUM"))
    dots_ps = ctx.enter_context(tc.tile_pool(name="dots_ps", bufs=1, space="PSUM"))
    sm_ps = ctx.enter_context(tc.tile_pool(name="sm_ps", bufs=2, space="PSUM"))

    consts = ctx.enter_context(tc.tile_pool(name="consts", bufs=1))
    wload = ctx.enter_context(tc.tile_pool(name="wload", bufs=2))
    x_pool = ctx.enter_context(tc.tile_pool(name="x_pool", bufs=2 * NT))
    xT_pool = ctx.enter_context(tc.tile_pool(name="xT_pool", bufs=1))
    kv_sb = ctx.enter_context(tc.tile_pool(name="kv_sb", bufs=2 * NT))
    stats_pool = ctx.enter_context(tc.tile_pool(name="stats_pool", bufs=2))
    small_pool = ctx.enter_context(tc.tile_pool(name="small_pool", bufs=2))
    p2_pool = ctx.enter_context(tc.tile_pool(name="p2_pool", bufs=4))
    out_pool = ctx.enter_context(tc.tile_pool(name="out_pool", bufs=4))
    dram = ctx.enter_context(tc.tile_pool(name="dram", bufs=1, space="DRAM"))

    # ---------------- constants / weights ----------------
    ident = consts.tile([128, 128], F16, tag="ident")
    make_identity(nc, ident[:])

    # C' = (I - J/64)/sqrt(N), blockdiag over the head pair: zero cross blocks
    rtn = float(np.sqrt(float(n_full)))
    cmat = consts.tile([128, 128], F16, tag="cmat")
    nc.gpsimd.memset(cmat[:], 0.0)
    nc.gpsimd.memset(cmat[0:64, 0:64], -1.0 / (64.0 * rtn))
    nc.gpsimd.memset(cmat[64:128, 64:128], -1.0 / (64.0 * rtn))
    nc.vector.scalar_tensor_tensor(
        cmat[:], ident[:], 1.0 / rtn, cmat[:],
        op0=mybir.AluOpType.mult, op1=mybir.AluOpType.add)

    identf = consts.tile([128, 128], F32, tag="identf")
    make_identity(nc, identf[:])

    # transposed W{k,v}: wkv_t[cs][c, ei*128+r] = W{k,v}[ei*128+r, cs*128+c]
    wkv_t = [consts.tile([128, 2 * INNER], F16, tag=f"wkv_t{c}", name=f"wkv_t{c}")
             for c in range(2)]
    for wi, w_d in enumerate((wk_d, wv_d)):
        for ei in range(INNER // 128):
            wn = wload.tile([128, CIN], F16, tag="wn")
            nc.gpsimd.dma_start(wn[:], w_d[ei * 128:(ei + 1) * 128, :])
            for cs in range(2):
                ps = xpose_ps.tile([128, 256], F16, tag="xp", name="ps")
                nc.tensor.transpose(ps[:, 0:128], wn[:, cs * 128:(cs + 1) * 128],
                                    ident[:])
                nc.scalar.copy(
                    wkv_t[cs][:, wi * INNER + ei * 128:wi * INNER + (ei + 1) * 128],
                    ps[:, 0:128])

    # Wo^T: wo_t[j][e, oi*128+o] = Wo[oi*128+o, j*128+e]
    wo_t = [consts.tile([128, COUT], F16, tag=f"wo_t{j}", name=f"wo_t{j}")
            for j in range(4)]
    for oi in range(COUT // 128):
        won = wload.tile([128, INNER], F16, tag="wn2")
        nc.gpsimd.dma_start(won[:], wo_d[oi * 128:(oi + 1) * 128, :])
        for j in range(4):
            ps = xpose_ps.tile([128, 256], F16, tag="xp", name="ps")
            nc.tensor.transpose(ps[:, 0:128], won[:, j * 128:(j + 1) * 128],
                                ident[:])
            nc.scalar.copy(wo_t[j][:, oi * 128:(oi + 1) * 128], ps[:, 0:128])

    # per-head sum weights: m16_t[cs][c, g] = sum_d wkv_t[cs][c, (g d)]
    m16_t = []
    for cs in range(2):
        msf = wload.tile([128, 16], F32, tag="msf", name=f"msf{cs}")
        nc.vector.reduce_sum(msf[:], wkv_t[cs][:].rearrange(
            "p (g d) -> p g d", d=D), axis=mybir.AxisListType.X)
        m16 = consts.tile([128, 16], F16, tag=f"m16_{cs}", name=f"m16_{cs}")
        nc.scalar.copy(m16[:], msf[:])
        m16_t.append(m16)

    # Wq natural rows (per pair) and Wq' = C' @ Wq_pair
    wq_nat = []
    wq_p = []
    for p in range(NPAIR):
        wqn = consts.tile([128, CIN], F16, tag=f"wqn{p}", name=f"wqn{p}")
        nc.gpsimd.dma_start(wqn[:], wq_d[p * 128:(p + 1) * 128, :])
        wq_nat.append(wqn)
    for p in range(NPAIR):
        ps = sm_ps.tile([128, 256], F32, tag="sm", name="ps")
        nc.tensor.matmul(ps[:], cmat[:], wq_nat[p][:], start=True, stop=True)
        wqp = consts.tile([128, CIN], F16, tag=f"wqp{p}", name=f"wqp{p}")
        nc.scalar.copy(wqp[:], ps[:])
        wq_p.append(wqp)

    # bias broadcast [128, COUT] via ones outer product
    bo_sb = consts.tile([1, COUT], F32, tag="bo_sb")
    nc.sync.dma_start(bo_sb[:], bo_d[:])
    ones1 = consts.tile([1, 128], F32, tag="ones1")
    nc.gpsimd.memset(ones1[:], 1.0)
    bias_ps = sm_ps.tile([128, 256], F32, tag="sm", name="bias_ps")
    nc.tensor.matmul(bias_ps[:], ones1[:], bo_sb[:], start=True, stop=True)
    bias_bc = consts.tile([128, COUT], F32, tag="bias_bc")
    nc.scalar.copy(bias_bc[:], bias_ps[:])

    # M_z staging tiles (pre-zeroed; only diag blocks ever written)
    mz_all = {}
    for b in range(B):
        for p in range(NPAIR):
            mz = consts.tile([128, 128], F16, tag=f"mz{b}_{p}", name=f"mz{b}_{p}")
            nc.gpsimd.memset(mz[:], 0.0)
            mz_all[(b, p)] = mz

    # dots staging: [128, B * NPAIR * 64]
    dcols = B * NPAIR * 64
    dots_l = consts.tile([128, dcols], F32, tag="dots_l")
    dots_a = consts.tile([128, dcols], F32, tag="dots_a")

    xT_all = {}   # (b, cs) -> [128, n_chunk] f16
    kraw_all = {}  # (b, nt) -> [128, 1024] f16 (k | v)
    kt_all = {}
    cc_out_all = {}
    a_all = {}

    # ---------------- phase 1a: x load/transpose, kv proj, stats ----------------
    def phase1a(b):
        x_ts = []
        for nt in range(NT):
            x_t = x_pool.tile([128, CIN], F32, tag="x", name="x_t")
            nc.sync.dma_start(x_t[:], x_d[b, nt * 128:(nt + 1) * 128, :])
            x_ts.append(x_t)
        for cs in range(2):
            xT = xT_pool.tile([128, n_chunk], F16, tag=f"xT{b}_{cs}",
                              name=f"xT{b}_{cs}")
            xT_all[(b, cs)] = xT
            for ntp in range(NT // 2):
                ps = xpose_ps.tile([128, 256], F32, tag="xp", name="ps")
                nc.tensor.transpose(
                    ps[:, 0:128], x_ts[2 * ntp][:, cs * 128:(cs + 1) * 128],
                    identf[:])
                nc.tensor.transpose(
                    ps[:, 128:256], x_ts[2 * ntp + 1][:, cs * 128:(cs + 1) * 128],
                    identf[:])
                nc.scalar.copy(xT[:, ntp * 256:(ntp + 1) * 256], ps[:])

        # S1 = sum_d k|v (from the PE mean-matmul), S2 = sum_d (k|v)^2
        s1 = stats_pool.tile([128, NT, 16], F32, tag="s1", name="s1")
        s2 = stats_pool.tile([128, NT, 16], F32, tag="s2", name="s2")
        for nt in range(NT):
            kps = kv_ps.tile([128, 512], F32, tag="kv", name="kps")
            vps = kv_ps.tile([128, 512], F32, tag="kv", name="vps")
            for cs in range(2):
                xT_sl = xT_all[(b, cs)][:, nt * 128:(nt + 1) * 128]
                nc.tensor.matmul(kps[:], xT_sl, wkv_t[cs][:, :INNER],
                                 start=(cs == 0), stop=(cs == 1))
                nc.tensor.matmul(vps[:], xT_sl, wkv_t[cs][:, INNER:],
                                 start=(cs == 0), stop=(cs == 1))
            mps = sm_ps.tile([128, 256], F32, tag="sm", name="mps")
            for cs in range(2):
                xT_sl = xT_all[(b, cs)][:, nt * 128:(nt + 1) * 128]
                nc.tensor.matmul(mps[:, 0:16], xT_sl, m16_t[cs][:],
                                 start=(cs == 0), stop=(cs == 1))
            kvraw = kv_sb.tile([128, 1024], F16, tag="kvraw", name="kvraw")
            kraw_all[(b, nt)] = kvraw
            nc.scalar.copy(kvraw[:, 0:512], kps[:])
            nc.scalar.copy(kvraw[:, 512:1024], vps[:])
            nc.vector.tensor_copy(s1[:, nt, :], mps[:, 0:16])
            sq = kv_sb.tile([128, 1024], F16, tag="sq", bufs=3, name="sq")
            nc.vector.tensor_tensor(sq[:], kvraw[:], kvraw[:],
                                    op=mybir.AluOpType.mult)
            nc.vector.reduce_sum(s2[:, nt, :],
                                 sq.rearrange("p (g d) -> p g d", d=D),
                                 axis=mybir.AxisListType.X)

        # var*4096 = 64*S2 - S1^2 ;  a = 4096*rsqrt(prodk*prodv)
        t1 = small_pool.tile([128, NT, 16], F32, tag="t1")
        nc.vector.tensor_tensor(t1[:], s1[:], s1[:], op=mybir.AluOpType.mult)
        t2 = small_pool.tile([128, NT, 16], F32, tag="t2")
        nc.vector.scalar_tensor_tensor(
            t2[:], s2[:], 64.0, t1[:],
            op0=mybir.AluOpType.mult, op1=mybir.AluOpType.subtract)
        nc.vector.tensor_scalar_add(t2[:], t2[:], 4096.0 * EPS)
        prod = small_pool.tile([128, NT, 8], F32, tag="prod")
        nc.vector.tensor_tensor(prod[:], t2[:, :, 0:8], t2[:, :, 8:16],
                                op=mybir.AluOpType.mult)
        nc.scalar.activation(prod[:], prod[:], mybir.ActivationFunctionType.Sqrt)
        recp = small_pool.tile([128, NT, 8], F32, tag="recp")
        nc.vector.reciprocal(recp[:], prod[:])
        a_sc = small_pool.tile([128, NT, 8], F16, tag="asc")
        nc.vector.tensor_scalar_mul(a_sc[:], recp[:], 4096.0)
        a_all[b] = a_sc

        for nt in range(NT):
            kt = kv_sb.tile([128, 512], F16, tag="kt", name="kt")
            kt_all[(b, nt)] = kt
            nc.gpsimd.tensor_tensor(
                kt.rearrange("p (h d) -> p h d", d=D),
                kraw_all[(b, nt)][:, 0:512].rearrange("p (h d) -> p h d", d=D),
                a_sc[:, nt, :].broadcast_to([128, 8, D]),
                op=mybir.AluOpType.mult)

    # ---------------- dots + AllReduce ----------------
    def dots_ar(b):
        acc = dots_ps.tile([128, 512], F32, tag="dots", name="acc")
        for p in range(NPAIR):
            for nt in range(NT):
                nc.tensor.matmul(
                    acc[:, p * 128:(p + 1) * 128],
                    kt_all[(b, nt)][:, p * 128:(p + 1) * 128],
                    kraw_all[(b, nt)][:, 512 + p * 128:512 + (p + 1) * 128],
                    start=(nt == 0), stop=(nt == NT - 1))
        for p in range(NPAIR):
            col = (b * NPAIR + p) * 64
            nc.vector.tensor_copy(dots_l[0:64, col:col + 64],
                                  acc[0:64, p * 128:p * 128 + 64])
            nc.vector.tensor_copy(dots_l[64:128, col:col + 64],
                                  acc[64:128, p * 128 + 64:(p + 1) * 128])
        bcols = NPAIR * 64
        cc_in = dram.tile([128, bcols], F32, tag="cc_in", bufs=B, name=f"cc_in{b}")
        cc_out = dram.tile([128, bcols], F32, tag="cc_out", bufs=B, name=f"cc_out{b}")
        bsl = slice(b * bcols, (b + 1) * bcols)
        nc.gpsimd.dma_start(cc_in[:], dots_l[:, bsl])
        nc.gpsimd.collective_compute(
            "AllReduce", mybir.AluOpType.add,
            replica_groups=[list(range(ncores))],
            ins=[cc_in.opt()], outs=[cc_out.opt()])
        cc_out_all[b] = cc_out

    # ---------------- phase 2: fold dots into weights, out proj ----------------
    def phase2(b):
        bcols = NPAIR * 64
        bsl = slice(b * bcols, (b + 1) * bcols)
        nc.sync.dma_start(dots_a[:, bsl], cc_out_all[b][:])
        wt_sb = []
        for p in range(NPAIR):
            col = (b * NPAIR + p) * 64
            mz = mz_all[(b, p)]
            nc.scalar.copy(mz[0:64, 0:64], dots_a[0:64, col:col + 64])
            nc.scalar.copy(mz[64:128, 64:128], dots_a[64:128, col:col + 64])
            ps = sm_ps.tile([128, 256], F32, tag="sm", name="ps_r")
            nc.tensor.matmul(ps[:], mz[:], wq_p[p][:], start=True, stop=True)
            r_sb = p2_pool.tile([128, 256], F16, tag="r_sb", name="r_sb")
            nc.scalar.copy(r_sb[:], ps[:])
            ps2 = sm_ps.tile([128, 256], F32, tag="sm", name="ps_w")
            nc.tensor.matmul(ps2[:], cmat[:], r_sb[:], start=True, stop=True)
            wt = p2_pool.tile([128, 256], F16, tag="wt", bufs=2 * NPAIR, name="wt")
            nc.scalar.copy(wt[:], ps2[:])
            wt_sb.append(wt)
        gt_sb = []
        for cs in range(2):
            psg = sm_ps.tile([128, 256], F32, tag="sm", name="ps_g")
            for j in range(NPAIR):
                nc.tensor.matmul(psg[:], wt_sb[j][:, cs * 128:(cs + 1) * 128],
                                 wo_t[j][:], start=(j == 0), stop=(j == NPAIR - 1))
            gt = p2_pool.tile([128, 256], F16, tag="gt", name="gt")
            nc.scalar.copy(gt[:], psg[:])
            gt_sb.append(gt)
        for nt in range(NT):
            pso = sm_ps.tile([128, 256], F32, tag="sm", name="ps_o")
            for cs in range(2):
                nc.tensor.matmul(pso[:],
                                 xT_all[(b, cs)][:, nt * 128:(nt + 1) * 128],
                                 gt_sb[cs][:], start=(cs == 0), stop=(cs == 1))
            osb = out_pool.tile([128, COUT], F32, tag="osb", name="osb")
            nc.vector.tensor_tensor(osb[:], pso[:], bias_bc[:],
                                    op=mybir.AluOpType.add)
            nc.sync.dma_start(out_d[b, nt * 128:(nt + 1) * 128, :], osb[:])

    # schedule: keep PE fed; dots(b) emitted after phase1a(b+1) so the
    # normalization (vector) of batch b overlaps the projections of b+1;
    # phase2(b) emitted ~2 phases after its AllReduce was issued.
    phase1a(0)
    phase1a(1)
    dots_ar(0)
    phase1a(2)
    dots_ar(1)
    phase2(0)
    phase1a(3)
    dots_ar(2)
    phase2(1)
    dots_ar(3)
    phase2(2)
    phase2(3)


_NC_CACHE = {}


def _get_nc(n_chunk, n_full, ncores):
    key = (n_chunk, n_full, ncores)
    if key not in _NC_CACHE:
        _NC_CACHE[key] = _build(n_chunk, n_full, ncores)
    return _NC_CACHE[key]


def _make_in_maps(u_x, Wq, Wk, Wv, Wo, bo, ncores):
    n = u_x.shape[1]
    n_chunk = n // ncores
    wq = np.ascontiguousarray(np.asarray(Wq, np.float32))
    wk = np.ascontiguousarray(np.asarray(Wk, np.float32))
    wv = np.ascontiguousarray(np.asarray(Wv, np.float32))
    wo = np.ascontiguousarray(np.asarray(Wo, np.float32))
    bo2 = np.ascontiguousarray(np.asarray(bo, np.float32).reshape(1, -1))
    u_x = np.asarray(u_x, np.float32)
    maps = []
    for c in range(ncores):
        maps.append({
            "x": np.ascontiguousarray(u_x[:, c * n_chunk:(c + 1) * n_chunk, :]),
            "wq": wq, "wk": wk, "wv": wv, "wo": wo, "bo": bo2,
        })
    return maps, n_chunk


def _install_ntff_hook():
    """Provide antenv.axon_hooks (missing in this image) so trace=True works."""
    import types
    try:
        from antenv.axon_hooks import get_axon_ntff_profile_hook  # noqa: F401
        return  # real module present
    except ImportError:
        pass
    try:
        import antenv
        mod = types.ModuleType("antenv.axon_hooks")
        _state = {"hook": None}
        mod.set_axon_ntff_profile_hook = lambda h: _state.__setitem__("hook", h)
        mod.get_axon_ntff_profile_hook = lambda: _state["hook"]
        sys.modules["antenv.axon_hooks"] = mod
        antenv.axon_hooks = mod
        boot_dir = "/root/.axon_site/trn_agent_boot"
        if boot_dir not in sys.path and os.path.isdir(boot_dir):
            sys.path.insert(0, boot_dir)
        import trn_boot
        so_path = "/opt/axon/libaxon_pjrt.so"
        if os.path.exists(so_path):
            hook = trn_boot._ntff_profile_via_ctypes(so_path)
            if hook is not None:
                mod.set_axon_ntff_profile_hook(hook)
    except Exception as e:  # tracing is best-effort; never break the run path
        print(f"ntff hook install failed: {e}", file=sys.stderr)


def run(u_x, Wq, Wk, Wv, Wo, bo, n_full=None, ncores=NCORES, trace=False,
        tmpdir=None):
    if trace:
        _install_ntff_hook()
    n = u_x.shape[1]
    if n_full is None:
        n_full = n
    in_maps, n_chunk = _make_in_maps(u_x, Wq, Wk, Wv, Wo, bo, ncores)
    nc = _get_nc(n_chunk, n_full, ncores)
    res = run_bass_kernel_spmd(nc, in_maps, list(range(ncores)), trace=trace,
                               tmpdir=tmpdir)
    outs = [np.asarray(res.results[c]["out"]) for c in range(ncores)]
    full = np.concatenate(outs, axis=1).astype(np.float32)
    return full, res


def kernel(u_x, pos_x=None, Wq=None, Wk=None, Wv=None, Wo=None, bo=None):
    full, _ = run(np.asarray(u_x, np.float32), Wq, Wk, Wv, Wo, bo)
    return full


# revision 10
# speedup vs baseline: 1.3085x; 1.1661x over previous
"""Trainium2 Bass kernel for nn_AttentionKernelIntegral (linear attention).

Math (per batch b, head h):
    q = x @ Wq^T                      [N, 512]  (no norm)
    k = inorm(x @ Wk^T)               per-(n,h) mean/var over d=64, biased
    v = inorm(x @ Wv^T)
    dots_h = k_h^T v_h                [64, 64]  (contract over ALL N)
    u_h = q_h @ dots_h / N
    out = u @ Wo^T + bo               [N, 256]

Key algebraic restructure vs the straightforward version:
  - Mean-centering over d is right-multiplication by C = I - J/64, so
        dots_h = C (K_h^T diag(a) V_h) C,   a_n = rsqrt((var_k+eps)(var_v+eps))
    i.e. only the per-row scale a touches the big [N,512] tensors (folded
    into k only); centering moves to tiny [64,64] matmuls AFTER the
    AllReduce of the un-centered M = K^T diag(a) V.
  - u = Q dots and out = u Wo^T fold into per-batch weights:
        G_b = Wo blockdiag(dots_b / N) Wq ;  out = X G_b^T + bo
    eliminating the q projection staging and the u apply entirely.

Sharding: rows (N) split across 8 cores; only [B,H,64,64] M is all-reduced.
Per-core dataflow per batch (n_chunk rows):
    x --gpsimd cast DMA--> f16 --PE transpose--> xT [256, n_chunk]
    k,v = xT^T @ W{k,v}^T (f16 PE, PSUM) --scalar/gpsimd--> f16 kraw/vraw
    bn_stats(kraw/vraw) -> var -> a;  kt = kraw * a (vector)
    M(pair p) += kt^T vraw  (PE, accumulated over row tiles)
    AllReduce(M diag blocks);  then per pair:
      R = M_z^T (C' Wq),  Wt = C' R,  (C' = (I-J/64)/sqrt(N))
      G^T = sum_j Wt_j^T Wo^T_j,  out = X G^T + bo
"""

import os
import sys

import numpy as np

for _p in ("/opt/trn_rl_repo", os.path.expanduser("~/.axon_site/_ro/trn_rl_repo")):
    if os.path.isdir(_p) and _p not in sys.path:
        sys.path.insert(0, _p)

from contextlib import ExitStack

import concourse.bass as bass
import concourse.mybir as mybir
import concourse.tile as tile
from concourse import bacc
from concourse.bass_utils import run_bass_kernel_spmd
from concourse.masks import make_identity

F32 = mybir.dt.float32
F16 = mybir.dt.float16

B, CIN = 4, 256
H, D = 8, 64
INNER, COUT = 512, 256
EPS = 1e-5
NCORES = 8
N_FULL = 8192
NPAIR = H // 2  # head pairs


def _build(n_chunk, n_full=N_FULL, ncores=NCORES):
    """Build the per-core SPMD Bass program. n_chunk rows per batch per core."""
    NT = n_chunk // 128  # 128-row tiles per batch
    nc = bacc.Bacc(
        "TRN2", target_bir_lowering=False, debug=False, num_devices=ncores)

    x_d = nc.declare_dram_parameter("x", [B, n_chunk, CIN], F32, isOutput=False)
    wq_d = nc.declare_dram_parameter("wq", [INNER, CIN], F32, isOutput=False)
    wk_d = nc.declare_dram_parameter("wk", [INNER, CIN], F32, isOutput=False)
    wv_d = nc.declare_dram_parameter("wv", [INNER, CIN], F32, isOutput=False)
    wo_d = nc.declare_dram_parameter("wo", [COUT, INNER], F32, isOutput=False)
    bo_d = nc.declare_dram_parameter("bo", [1, COUT], F32, isOutput=False)
    out_d = nc.declare_dram_parameter("out", [B, n_chunk, COUT], F32, isOutput=True)

    with ExitStack() as ctx:
        tc = ctx.enter_context(tile.TileContext(nc))
        _body(ctx, tc, nc, NT, n_full, ncores,
              x_d, wq_d, wk_d, wv_d, wo_d, bo_d, out_d)
    nc.compile()
    return nc


def _bn_stats_seg(nc, out, in_):
    """bn_stats with the segmented (3D) access pattern preserved.

    nc.vector.bn_stats lowers with opt=True which merges contiguous free
    dims, collapsing [p, h, d] to [p, h*d] and producing a single segment.
    Lower with opt=False so each h gets its own 6-tuple of stats.
    """
    eng = nc.vector
    return eng.add_instruction(
        mybir.InstBNStats(
            name=nc.get_next_instruction_name(),
            ins=[eng.lower_ap(in_, opt=False)],
            outs=[eng.lower_ap(out, opt=False)],
        )
    )


def _body(ctx, tc, nc, NT, n_full, ncores,
          x_d, wq_d, wk_d, wv_d, wo_d, bo_d, out_d):
    n_chunk = NT * 128

    # ---------------- pools ----------------
    # PSUM budget (8 banks): xp(2) + kv(3) + dots(1) + sm(2) = 8
    xpose_ps = ctx.enter_context(tc.tile_pool(name="xpose_ps", bufs=2, space="PSUM"))
    kv_ps = ctx.enter_context(tc.tile_pool(name="kv_ps", bufs=3, space="PSUM"))
    dots_ps = ctx.enter_context(tc.tile_pool(name="dots_ps", bufs=1, space="PSUM"))
    sm_ps = ctx.enter_context(tc.tile_pool(name="sm_ps", bufs=2, space="PSUM"))

    consts = ctx.enter_context(tc.tile_pool(name="consts", bufs=1))
    wload = ctx.enter_context(tc.tile_pool(name="wload", bufs=2))
    x_pool = ctx.enter_context(tc.tile_pool(name="x_pool", bufs=2 * NT))
    xT_pool = ctx.enter_context(tc.tile_pool(name="xT_pool", bufs=1))
    kv_sb = ctx.enter_context(tc.tile_pool(name="kv_sb", bufs=2 * NT))
    stats_pool = ctx.enter_context(tc.tile_pool(name="stats_pool", bufs=2))
    small_pool = ctx.enter_context(tc.tile_pool(name="small_pool", bufs=2))
    p2_pool = ctx.enter_context(tc.tile_pool(name="p2_pool", bufs=4))
    out_pool = ctx.enter_context(tc.tile_pool(name="out_pool", bufs=4))
    dram = ctx.enter_context(tc.tile_pool(name="dram", bufs=1, space="DRAM"))

    # ---------------- constants / weights ----------------
    ident = consts.tile([128, 128], F16, tag="ident")
    make_identity(nc, ident[:])

    # C' = (I - J/64)/sqrt(N), blockdiag over the head pair: zero cross blocks
    rtn = float(np.sqrt(float(n_full)))
    cmat = consts.tile([128, 128], F16, tag="cmat")
    nc.gpsimd.memset(cmat[:], 0.0)
    nc.gpsimd.memset(cmat[0:64, 0:64], -1.0 / (64.0 * rtn))
    nc.gpsimd.memset(cmat[64:128, 64:128], -1.0 / (64.0 * rtn))
    nc.vector.scalar_tensor_tensor(
        cmat[:], ident[:], 1.0 / rtn, cmat[:],
        op0=mybir.AluOpType.mult, op1=mybir.AluOpType.add)

    identf = consts.tile([128, 128], F32, tag="identf")
    make_identity(nc, identf[:])

    # transposed W{k,v}: wkv_t[cs][c, ei*128+r] = W{k,v}[ei*128+r, cs*128+c]
    wkv_t = [consts.tile([128, 2 * INNER], F16, tag=f"wkv_t{c}", name=f"wkv_t{c}")
             for c in range(2)]
    for wi, w_d in enumerate((wk_d, wv_d)):
        for ei in range(INNER // 128):
            wn = wload.tile([128, CIN], F16, tag="wn")
            nc.gpsimd.dma_start(wn[:], w_d[ei * 128:(ei + 1) * 128, :])
            for cs in range(2):
                ps = xpose_ps.tile([128, 256], F16, tag="xp", name="ps")
                nc.tensor.transpose(ps[:, 0:128], wn[:, cs * 128:(cs + 1) * 128],
                                    ident[:])
                nc.vector.tensor_copy(
                    wkv_t[cs][:, wi * INNER + ei * 128:wi * INNER + (ei + 1) * 128],
                    ps[:, 0:128])

    # Wo^T: wo_t[j][e, oi*128+o] = Wo[oi*128+o, j*128+e]
    wo_t = [consts.tile([128, COUT], F16, tag=f"wo_t{j}", name=f"wo_t{j}")
            for j in range(4)]
    for oi in range(COUT // 128):
        won = wload.tile([128, INNER], F16, tag="wn2")
        nc.gpsimd.dma_start(won[:], wo_d[oi * 128:(oi + 1) * 128, :])
        for j in range(4):
            ps = xpose_ps.tile([128, 256], F16, tag="xp", name="ps")
            nc.tensor.transpose(ps[:, 0:128], won[:, j * 128:(j + 1) * 128],
                                ident[:])
            nc.vector.tensor_copy(wo_t[j][:, oi * 128:(oi + 1) * 128], ps[:, 0:128])

    # per-head sum weights: m16_t[cs][c, g] = sum_d wkv_t[cs][c, (g d)]
    m16_t = []
    for cs in range(2):
        msf = wload.tile([128, 16], F32, tag="msf", name=f"msf{cs}")
        nc.vector.reduce_sum(msf[:], wkv_t[cs][:].rearrange(
            "p (g d) -> p g d", d=D), axis=mybir.AxisListType.X)
        m16 = consts.tile([128, 16], F16, tag=f"m16_{cs}", name=f"m16_{cs}")
        nc.scalar.copy(m16[:], msf[:])
        m16_t.append(m16)

    # Wq natural rows (per pair) and Wq' = C' @ Wq_pair
    wq_nat = []
    wq_p = []
    for p in range(NPAIR):
        wqn = consts.tile([128, CIN], F16, tag=f"wqn{p}", name=f"wqn{p}")
        nc.gpsimd.dma_start(wqn[:], wq_d[p * 128:(p + 1) * 128, :])
        wq_nat.append(wqn)
    for p in range(NPAIR):
        ps = sm_ps.tile([128, 256], F32, tag="sm", name="ps")
        nc.tensor.matmul(ps[:], cmat[:], wq_nat[p][:], start=True, stop=True)
        wqp = consts.tile([128, CIN], F16, tag=f"wqp{p}", name=f"wqp{p}")
        nc.vector.tensor_copy(wqp[:], ps[:])
        wq_p.append(wqp)

    # bias broadcast [128, COUT] via ones outer product
    bo_sb = consts.tile([1, COUT], F32, tag="bo_sb")
    nc.sync.dma_start(bo_sb[:], bo_d[:])
    ones1 = consts.tile([1, 128], F32, tag="ones1")
    nc.gpsimd.memset(ones1[:], 1.0)
    bias_ps = sm_ps.tile([128, 256], F32, tag="sm", name="bias_ps")
    nc.tensor.matmul(bias_ps[:], ones1[:], bo_sb[:], start=True, stop=True)
    bias_bc = consts.tile([128, COUT], F32, tag="bias_bc")
    nc.vector.tensor_copy(bias_bc[:], bias_ps[:])

    # M_z staging tiles (pre-zeroed; only diag blocks ever written)
    mz_all = {}
    for b in range(B):
        for p in range(NPAIR):
            mz = consts.tile([128, 128], F16, tag=f"mz{b}_{p}", name=f"mz{b}_{p}")
            nc.gpsimd.memset(mz[:], 0.0)
            mz_all[(b, p)] = mz

    # dots staging: [128, B * NPAIR * 64]
    dcols = B * NPAIR * 64
    dots_l = consts.tile([128, dcols], F32, tag="dots_l")
    dots_a = consts.tile([128, dcols], F32, tag="dots_a")

    xT_all = {}   # (b, cs) -> [128, n_chunk] f16
    kraw_all = {}  # (b, nt) -> [128, 1024] f16 (k | v)
    kt_all = {}
    cc_out_all = {}
    a_all = {}

    # ---------------- phase 1a ----------------
    x_tiles = {}  # b -> list of f16 [128, CIN] tiles

    def load_x(b):
        x_ts = []
        for nt in range(NT):
            x_t = x_pool.tile([128, CIN], F16, tag="x", name="x_t")
            nc.gpsimd.dma_start(x_t[:], x_d[b, nt * 128:(nt + 1) * 128, :])
            x_ts.append(x_t)
        x_tiles[b] = x_ts

    def phase1a(b):
        if b + 1 < B:
            load_x(b + 1)  # prefetch next batch on the gpsimd queue
        x_ts = x_tiles[b]
        for cs in range(2):
            xT = xT_pool.tile([128, n_chunk], F16, tag=f"xT{b}_{cs}",
                              name=f"xT{b}_{cs}")
            xT_all[(b, cs)] = xT
            for ntp in range(NT // 2):
                ps = xpose_ps.tile([128, 256], F16, tag="xp", name="ps")
                nc.tensor.transpose(
                    ps[:, 0:128], x_ts[2 * ntp][:, cs * 128:(cs + 1) * 128],
                    ident[:])
                nc.tensor.transpose(
                    ps[:, 128:256], x_ts[2 * ntp + 1][:, cs * 128:(cs + 1) * 128],
                    ident[:])
                nc.scalar.copy(xT[:, ntp * 256:(ntp + 1) * 256], ps[:])

        # S1 = sum_d k|v (from the PE mean-matmul), S2 = sum_d (k|v)^2
        s1 = stats_pool.tile([128, NT, 16], F32, tag="s1", name="s1")
        NT2 = NT // 2

        def stats(nt):
            kps = kv_ps.tile([128, 512], F32, tag="kv", name="kps")
            vps = kv_ps.tile([128, 512], F32, tag="kv", name="vps")
            for cs in range(2):
                xT_sl = xT_all[(b, cs)][:, nt * 128:(nt + 1) * 128]
                nc.tensor.matmul(kps[:], xT_sl, wkv_t[cs][:, :INNER],
                                 start=(cs == 0), stop=(cs == 1))
                nc.tensor.matmul(vps[:], xT_sl, wkv_t[cs][:, INNER:],
                                 start=(cs == 0), stop=(cs == 1))
            mps = sm_ps.tile([128, 256], F32, tag="sm", name="mps")
            for cs in range(2):
                xT_sl = xT_all[(b, cs)][:, nt * 128:(nt + 1) * 128]
                nc.tensor.matmul(mps[:, 0:16], xT_sl, m16_t[cs][:],
                                 start=(cs == 0), stop=(cs == 1))
            kvraw = kv_sb.tile([128, 1024], F16, tag="kvraw", name="kvraw")
            kraw_all[(b, nt)] = kvraw
            nc.scalar.copy(kvraw[:, 0:512], kps[:])
            nc.scalar.copy(kvraw[:, 512:1024], vps[:])
            nc.scalar.copy(s1[:, nt, :], mps[:, 0:16])
            sq = kv_sb.tile([128, 1024], F16, tag="sq", bufs=3, name="sq")
            nc.vector.tensor_tensor(sq[:], kvraw[:], kvraw[:],
                                    op=mybir.AluOpType.mult)
            with nc.allow_low_precision(reason="f16 S2 of ~64 is plenty"):
                nc.vector.reduce_sum(s2h[:, nt, :],
                                     sq.rearrange("p (g d) -> p g d", d=D),
                                     axis=mybir.AxisListType.X)

        s2h = stats_pool.tile([128, NT, 16], F16, tag="s2h", name="s2h")

        def combine_apply(w):
            sl = slice(w * NT2, (w + 1) * NT2)
            # var*4096 = 64*S2 - S1^2 ;  a = 4096*rsqrt(prodk*prodv)
            t1 = small_pool.tile([128, NT2, 16], F32, tag=f"t1{w}")
            nc.vector.tensor_tensor(t1[:], s1[:, sl, :], s1[:, sl, :],
                                    op=mybir.AluOpType.mult)
            t2 = small_pool.tile([128, NT2, 16], F32, tag=f"t2{w}")
            nc.vector.scalar_tensor_tensor(
                t2[:], s2h[:, sl, :], 64.0, t1[:],
                op0=mybir.AluOpType.mult, op1=mybir.AluOpType.subtract)
            nc.vector.tensor_scalar_add(t2[:], t2[:], 4096.0 * EPS)
            prod = small_pool.tile([128, NT2, 8], F32, tag=f"prod{w}")
            nc.vector.tensor_tensor(prod[:], t2[:, :, 0:8], t2[:, :, 8:16],
                                    op=mybir.AluOpType.mult)
            nc.scalar.activation(prod[:], prod[:],
                                 mybir.ActivationFunctionType.Sqrt)
            recp = small_pool.tile([128, NT2, 8], F32, tag=f"recp{w}")
            nc.vector.reciprocal(recp[:], prod[:])
            a_sc = small_pool.tile([128, NT2, 8], F16, tag=f"asc{w}")
            nc.vector.tensor_scalar_mul(a_sc[:], recp[:], 4096.0)
            aexp = small_pool.tile([128, NT2, 512], F16, tag=f"aexp{w}")
            nc.vector.tensor_copy(
                aexp.rearrange("p t (h d) -> p t h d", d=D),
                a_sc.broadcast_to([128, NT2, 8, D]))
            for i in range(NT2):
                nt = w * NT2 + i
                kt = kv_sb.tile([128, 512], F16, tag="kt", name="kt")
                kt_all[(b, nt)] = kt
                nc.vector.tensor_tensor(kt[:], kraw_all[(b, nt)][:, 0:512],
                                        aexp[:, i, :], op=mybir.AluOpType.mult)

        for nt in range(NT2):
            stats(nt)
        combine_apply(0)
        for nt in range(NT2, NT):
            stats(nt)
        combine_apply(1)

    # ---------------- dots + AllReduce ----------------
    # ---------------- dots + AllReduce ----------------
    def dots_ar(b):
        acc = dots_ps.tile([128, 512], F32, tag="dots", name="acc")
        for p in range(NPAIR):
            for nt in range(NT):
                nc.tensor.matmul(
                    acc[:, p * 128:(p + 1) * 128],
                    kt_all[(b, nt)][:, p * 128:(p + 1) * 128],
                    kraw_all[(b, nt)][:, 512 + p * 128:512 + (p + 1) * 128],
                    start=(nt == 0), stop=(nt == NT - 1))
        for p in range(NPAIR):
            col = (b * NPAIR + p) * 64
            nc.scalar.copy(dots_l[0:64, col:col + 64],
                           acc[0:64, p * 128:p * 128 + 64])
            nc.scalar.copy(dots_l[64:128, col:col + 64],
                           acc[64:128, p * 128 + 64:(p + 1) * 128])
        bcols = NPAIR * 64
        cc_in = dram.tile([128, bcols], F32, tag="cc_in", bufs=B, name=f"cc_in{b}")
        cc_out = dram.tile([128, bcols], F32, tag="cc_out", bufs=B, name=f"cc_out{b}")
        bsl = slice(b * bcols, (b + 1) * bcols)
        nc.gpsimd.dma_start(cc_in[:], dots_l[:, bsl])
        nc.gpsimd.collective_compute(
            "AllReduce", mybir.AluOpType.add,
            replica_groups=[list(range(ncores))],
            ins=[cc_in.opt()], outs=[cc_out.opt()])
        cc_out_all[b] = cc_out

    # ---------------- phase 2: fold dots into weights, out proj ----------------
    def phase2(b):
        bcols = NPAIR * 64
        bsl = slice(b * bcols, (b + 1) * bcols)
        nc.sync.dma_start(dots_a[:, bsl], cc_out_all[b][:])
        wt_sb = []
        for p in range(NPAIR):
            col = (b * NPAIR + p) * 64
            mz = mz_all[(b, p)]
            nc.scalar.copy(mz[0:64, 0:64], dots_a[0:64, col:col + 64])
            nc.scalar.copy(mz[64:128, 64:128], dots_a[64:128, col:col + 64])
            ps = sm_ps.tile([128, 256], F32, tag="sm", name="ps_r")
            nc.tensor.matmul(ps[:], mz[:], wq_p[p][:], start=True, stop=True)
            r_sb = p2_pool.tile([128, 256], F16, tag="r_sb", name="r_sb")
            nc.scalar.copy(r_sb[:], ps[:])
            ps2 = sm_ps.tile([128, 256], F32, tag="sm", name="ps_w")
            nc.tensor.matmul(ps2[:], cmat[:], r_sb[:], start=True, stop=True)
            wt = p2_pool.tile([128, 256], F16, tag="wt", bufs=2 * NPAIR, name="wt")
            nc.scalar.copy(wt[:], ps2[:])
            wt_sb.append(wt)
        gt_sb = []
        for cs in range(2):
            psg = sm_ps.tile([128, 256], F32, tag="sm", name="ps_g")
            for j in range(NPAIR):
                nc.tensor.matmul(psg[:], wt_sb[j][:, cs * 128:(cs + 1) * 128],
                                 wo_t[j][:], start=(j == 0), stop=(j == NPAIR - 1))
            gt = p2_pool.tile([128, 256], F16, tag="gt", name="gt")
            nc.scalar.copy(gt[:], psg[:])
            gt_sb.append(gt)
        for nt in range(NT):
            pso = sm_ps.tile([128, 256], F32, tag="sm", name="ps_o")
            for cs in range(2):
                nc.tensor.matmul(pso[:],
                                 xT_all[(b, cs)][:, nt * 128:(nt + 1) * 128],
                                 gt_sb[cs][:], start=(cs == 0), stop=(cs == 1))
            osb = out_pool.tile([128, COUT], F32, tag="osb", name="osb")
            nc.vector.tensor_tensor(osb[:], pso[:], bias_bc[:],
                                    op=mybir.AluOpType.add)
            nc.sync.dma_start(out_d[b, nt * 128:(nt + 1) * 128, :], osb[:])

    # schedule: keep PE fed; dots(b) emitted after phase1a(b+1) so the
    # normalization (vector) of batch b overlaps the projections of b+1;
    # phase2(b) emitted ~2 phases after its AllReduce was issued.
    load_x(0)
    phase1a(0)
    dots_ar(0)
    phase1a(1)
    dots_ar(1)
    phase1a(2)
    dots_ar(2)
    phase2(0)
    phase1a(3)
    dots_ar(3)
    phase2(1)
    phase2(2)
    phase2(3)


_NC_CACHE = {}


def _get_nc(n_chunk, n_full, ncores):
    key = (n_chunk, n_full, ncores)
    if key not in _NC_CACHE:
        _NC_CACHE[key] = _build(n_chunk, n_full, ncores)
    return _NC_CACHE[key]


def _make_in_maps(u_x, Wq, Wk, Wv, Wo, bo, ncores):
    n = u_x.shape[1]
    n_chunk = n // ncores
    wq = np.ascontiguousarray(np.asarray(Wq, np.float32))
    wk = np.ascontiguousarray(np.asarray(Wk, np.float32))
    wv = np.ascontiguousarray(np.asarray(Wv, np.float32))
    wo = np.ascontiguousarray(np.asarray(Wo, np.float32))
    bo2 = np.ascontiguousarray(np.asarray(bo, np.float32).reshape(1, -1))
    u_x = np.asarray(u_x, np.float32)
    maps = []
    for c in range(ncores):
        maps.append({
            "x": np.ascontiguousarray(u_x[:, c * n_chunk:(c + 1) * n_chunk, :]),
            "wq": wq, "wk": wk, "wv": wv, "wo": wo, "bo": bo2,
        })
    return maps, n_chunk


def _install_ntff_hook():
    """Provide antenv.axon_hooks (missing in this image) so trace=True works."""
    import types
    try:
        from antenv.axon_hooks import get_axon_ntff_profile_hook  # noqa: F401
        return  # real module present
    except ImportError:
        pass
    try:
        import antenv
        mod = types.ModuleType("antenv.axon_hooks")
        _state = {"hook": None}
        mod.set_axon_ntff_profile_hook = lambda h: _state.__setitem__("hook", h)
        mod.get_axon_ntff_profile_hook = lambda: _state["hook"]
        sys.modules["antenv.axon_hooks"] = mod
        antenv.axon_hooks = mod
        boot_dir = "/root/.axon_site/trn_agent_boot"
        if boot_dir not in sys.path and os.path.isdir(boot_dir):
            sys.path.insert(0, boot_dir)
        import trn_boot
        so_path = "/opt/axon/libaxon_pjrt.so"
        if os.path.exists(so_path):
            hook = trn_boot._ntff_profile_via_ctypes(so_path)
            if hook is not None:
                mod.set_axon_ntff_profile_hook(hook)
    except Exception as e:  # tracing is best-effort; never break the run path
        print(f"ntff hook install failed: {e}", file=sys.stderr)


def run(u_x, Wq, Wk, Wv, Wo, bo, n_full=None, ncores=NCORES, trace=False,
        tmpdir=None):
    if trace:
        _install_ntff_hook()
    n = u_x.shape[1]
    if n_full is None:
        n_full = n
    in_maps, n_chunk = _make_in_maps(u_x, Wq, Wk, Wv, Wo, bo, ncores)
    nc = _get_nc(n_chunk, n_full, ncores)
    res = run_bass_kernel_spmd(nc, in_maps, list(range(ncores)), trace=trace,
                               tmpdir=tmpdir)
    outs = [np.asarray(res.results[c]["out"]) for c in range(ncores)]
    full = np.concatenate(outs, axis=1).astype(np.float32)
    return full, res


def kernel(u_x, pos_x=None, Wq=None, Wk=None, Wv=None, Wo=None, bo=None):
    full, _ = run(np.asarray(u_x, np.float32), Wq, Wk, Wv, Wo, bo)
    return full


# revision 11
# speedup vs baseline: 1.3782x; 1.0532x over previous
"""Trainium2 Bass kernel for nn_AttentionKernelIntegral (linear attention).

Math (per batch b, head h):
    q = x @ Wq^T                      [N, 512]  (no norm)
    k = inorm(x @ Wk^T)               per-(n,h) mean/var over d=64, biased
    v = inorm(x @ Wv^T)
    dots_h = k_h^T v_h                [64, 64]  (contract over ALL N)
    u_h = q_h @ dots_h / N
    out = u @ Wo^T + bo               [N, 256]

Key algebraic restructure vs the straightforward version:
  - Mean-centering over d is right-multiplication by C = I - J/64, so
        dots_h = C (K_h^T diag(a) V_h) C,   a_n = rsqrt((var_k+eps)(var_v+eps))
    i.e. only the per-row scale a touches the big [N,512] tensors (folded
    into k only); centering moves to tiny [64,64] matmuls AFTER the
    AllReduce of the un-centered M = K^T diag(a) V.
  - u = Q dots and out = u Wo^T fold into per-batch weights:
        G_b = Wo blockdiag(dots_b / N) Wq ;  out = X G_b^T + bo
    eliminating the q projection staging and the u apply entirely.

Sharding: rows (N) split across 8 cores; only [B,H,64,64] M is all-reduced.
Per-core dataflow per batch (n_chunk rows):
    x --gpsimd cast DMA--> f16 --PE transpose--> xT [256, n_chunk]
    k,v = xT^T @ W{k,v}^T (f16 PE, PSUM) --scalar/gpsimd--> f16 kraw/vraw
    bn_stats(kraw/vraw) -> var -> a;  kt = kraw * a (vector)
    M(pair p) += kt^T vraw  (PE, accumulated over row tiles)
    AllReduce(M diag blocks);  then per pair:
      R = M_z^T (C' Wq),  Wt = C' R,  (C' = (I-J/64)/sqrt(N))
      G^T = sum_j Wt_j^T Wo^T_j,  out = X G^T + bo
"""

import os
import sys

import numpy as np

for _p in ("/opt/trn_rl_repo", os.path.expanduser("~/.axon_site/_ro/trn_rl_repo")):
    if os.path.isdir(_p) and _p not in sys.path:
        sys.path.insert(0, _p)

from contextlib import ExitStack

import concourse.bass as bass
import concourse.mybir as mybir
import concourse.tile as tile
from concourse import bacc
from concourse.bass_utils import run_bass_kernel_spmd
from concourse.masks import make_identity

F32 = mybir.dt.float32
F16 = mybir.dt.float16

B, CIN = 4, 256
H, D = 8, 64
INNER, COUT = 512, 256
EPS = 1e-5
NCORES = 8
N_FULL = 8192
NPAIR = H // 2  # head pairs


def _build(n_chunk, n_full=N_FULL, ncores=NCORES):
    """Build the per-core SPMD Bass program. n_chunk rows per batch per core."""
    NT = n_chunk // 128  # 128-row tiles per batch
    nc = bacc.Bacc(
        "TRN2", target_bir_lowering=False, debug=False, num_devices=ncores)

    x_d = nc.declare_dram_parameter("x", [B, n_chunk, CIN], F32, isOutput=False)
    wq_d = nc.declare_dram_parameter("wq", [INNER, CIN], F32, isOutput=False)
    wk_d = nc.declare_dram_parameter("wk", [INNER, CIN], F32, isOutput=False)
    wv_d = nc.declare_dram_parameter("wv", [INNER, CIN], F32, isOutput=False)
    wo_d = nc.declare_dram_parameter("wo", [COUT, INNER], F32, isOutput=False)
    bo_d = nc.declare_dram_parameter("bo", [1, COUT], F32, isOutput=False)
    out_d = nc.declare_dram_parameter("out", [B, n_chunk, COUT], F32, isOutput=True)

    with ExitStack() as ctx:
        tc = ctx.enter_context(tile.TileContext(nc))
        _body(ctx, tc, nc, NT, n_full, ncores,
              x_d, wq_d, wk_d, wv_d, wo_d, bo_d, out_d)
    nc.compile()
    return nc


def _bn_stats_seg(nc, out, in_):
    """bn_stats with the segmented (3D) access pattern preserved.

    nc.vector.bn_stats lowers with opt=True which merges contiguous free
    dims, collapsing [p, h, d] to [p, h*d] and producing a single segment.
    Lower with opt=False so each h gets its own 6-tuple of stats.
    """
    eng = nc.vector
    return eng.add_instruction(
        mybir.InstBNStats(
            name=nc.get_next_instruction_name(),
            ins=[eng.lower_ap(in_, opt=False)],
            outs=[eng.lower_ap(out, opt=False)],
        )
    )


def _body(ctx, tc, nc, NT, n_full, ncores,
          x_d, wq_d, wk_d, wv_d, wo_d, bo_d, out_d):
    n_chunk = NT * 128

    # ---------------- pools ----------------
    # PSUM budget (8 banks): xp(2) + kv(3) + dots(1) + sm(2) = 8
    xpose_ps = ctx.enter_context(tc.tile_pool(name="xpose_ps", bufs=2, space="PSUM"))
    kv_ps = ctx.enter_context(tc.tile_pool(name="kv_ps", bufs=3, space="PSUM"))
    dots_ps = ctx.enter_context(tc.tile_pool(name="dots_ps", bufs=1, space="PSUM"))
    sm_ps = ctx.enter_context(tc.tile_pool(name="sm_ps", bufs=2, space="PSUM"))

    consts = ctx.enter_context(tc.tile_pool(name="consts", bufs=1))
    wload = ctx.enter_context(tc.tile_pool(name="wload", bufs=2))
    x_pool = ctx.enter_context(tc.tile_pool(name="x_pool", bufs=2 * NT))
    xT_pool = ctx.enter_context(tc.tile_pool(name="xT_pool", bufs=1))
    kv_sb = ctx.enter_context(tc.tile_pool(name="kv_sb", bufs=2 * NT))
    stats_pool = ctx.enter_context(tc.tile_pool(name="stats_pool", bufs=2))
    small_pool = ctx.enter_context(tc.tile_pool(name="small_pool", bufs=2))
    p2_pool = ctx.enter_context(tc.tile_pool(name="p2_pool", bufs=4))
    out_pool = ctx.enter_context(tc.tile_pool(name="out_pool", bufs=4))
    dram = ctx.enter_context(tc.tile_pool(name="dram", bufs=1, space="DRAM"))

    # ---------------- warmup collective ----------------
    # The first collective pays a ~25us TOPSP cold-start; absorb it at t=0
    # with a dummy AllReduce so the real per-batch reductions run warm.
    wu = wload.tile([128, 64], F32, tag="wu", name="wu")
    nc.gpsimd.memset(wu[:], 0.0)
    cc_wu_in = dram.tile([128, 64], F32, tag="cc_wu_in", name="cc_wu_in")
    cc_wu_out = dram.tile([128, 64], F32, tag="cc_wu_out", name="cc_wu_out")
    nc.gpsimd.dma_start(cc_wu_in[:], wu[:])
    nc.gpsimd.collective_compute(
        "AllReduce", mybir.AluOpType.add,
        replica_groups=[list(range(ncores))],
        ins=[cc_wu_in.opt()], outs=[cc_wu_out.opt()])

    # ---------------- constants / weights ----------------
    ident = consts.tile([128, 128], F16, tag="ident")
    make_identity(nc, ident[:])

    # C' = (I - J/64)/sqrt(N), blockdiag over the head pair: zero cross blocks
    rtn = float(np.sqrt(float(n_full)))
    cmat = consts.tile([128, 128], F16, tag="cmat")
    nc.gpsimd.memset(cmat[:], 0.0)
    nc.gpsimd.memset(cmat[0:64, 0:64], -1.0 / (64.0 * rtn))
    nc.gpsimd.memset(cmat[64:128, 64:128], -1.0 / (64.0 * rtn))
    nc.vector.scalar_tensor_tensor(
        cmat[:], ident[:], 1.0 / rtn, cmat[:],
        op0=mybir.AluOpType.mult, op1=mybir.AluOpType.add)

    identf = consts.tile([128, 128], F32, tag="identf")
    make_identity(nc, identf[:])

    # transposed W{k,v}: wkv_t[cs][c, ei*128+r] = W{k,v}[ei*128+r, cs*128+c]
    wkv_t = [consts.tile([128, 2 * INNER], F16, tag=f"wkv_t{c}", name=f"wkv_t{c}")
             for c in range(2)]
    for wi, w_d in enumerate((wk_d, wv_d)):
        for ei in range(INNER // 128):
            wn = wload.tile([128, CIN], F16, tag="wn")
            nc.gpsimd.dma_start(wn[:], w_d[ei * 128:(ei + 1) * 128, :])
            for cs in range(2):
                ps = xpose_ps.tile([128, 256], F16, tag="xp", name="ps")
                nc.tensor.transpose(ps[:, 0:128], wn[:, cs * 128:(cs + 1) * 128],
                                    ident[:])
                nc.vector.tensor_copy(
                    wkv_t[cs][:, wi * INNER + ei * 128:wi * INNER + (ei + 1) * 128],
                    ps[:, 0:128])

    # Wo^T: wo_t[j][e, oi*128+o] = Wo[oi*128+o, j*128+e]
    wo_t = [consts.tile([128, COUT], F16, tag=f"wo_t{j}", name=f"wo_t{j}")
            for j in range(4)]
    for oi in range(COUT // 128):
        won = wload.tile([128, INNER], F16, tag="wn2")
        nc.gpsimd.dma_start(won[:], wo_d[oi * 128:(oi + 1) * 128, :])
        for j in range(4):
            ps = xpose_ps.tile([128, 256], F16, tag="xp", name="ps")
            nc.tensor.transpose(ps[:, 0:128], won[:, j * 128:(j + 1) * 128],
                                ident[:])
            nc.vector.tensor_copy(wo_t[j][:, oi * 128:(oi + 1) * 128], ps[:, 0:128])

    # per-head sum weights: m16_t[cs][c, g] = sum_d wkv_t[cs][c, (g d)]
    m16_t = []
    for cs in range(2):
        msf = wload.tile([128, 16], F32, tag="msf", name=f"msf{cs}")
        nc.vector.reduce_sum(msf[:], wkv_t[cs][:].rearrange(
            "p (g d) -> p g d", d=D), axis=mybir.AxisListType.X)
        m16 = consts.tile([128, 16], F16, tag=f"m16_{cs}", name=f"m16_{cs}")
        nc.scalar.copy(m16[:], msf[:])
        m16_t.append(m16)

    # Wq natural rows (per pair) and Wq' = C' @ Wq_pair
    wq_nat = []
    wq_p = []
    for p in range(NPAIR):
        wqn = consts.tile([128, CIN], F16, tag=f"wqn{p}", name=f"wqn{p}")
        nc.gpsimd.dma_start(wqn[:], wq_d[p * 128:(p + 1) * 128, :])
        wq_nat.append(wqn)
    for p in range(NPAIR):
        ps = sm_ps.tile([128, 256], F32, tag="sm", name="ps")
        nc.tensor.matmul(ps[:], cmat[:], wq_nat[p][:], start=True, stop=True)
        wqp = consts.tile([128, CIN], F16, tag=f"wqp{p}", name=f"wqp{p}")
        nc.vector.tensor_copy(wqp[:], ps[:])
        wq_p.append(wqp)

    # bias broadcast [128, COUT] via ones outer product
    bo_sb = consts.tile([1, COUT], F32, tag="bo_sb")
    nc.sync.dma_start(bo_sb[:], bo_d[:])
    ones1 = consts.tile([1, 128], F32, tag="ones1")
    nc.gpsimd.memset(ones1[:], 1.0)
    bias_ps = sm_ps.tile([128, 256], F32, tag="sm", name="bias_ps")
    nc.tensor.matmul(bias_ps[:], ones1[:], bo_sb[:], start=True, stop=True)
    bias_bc = consts.tile([128, COUT], F32, tag="bias_bc")
    nc.vector.tensor_copy(bias_bc[:], bias_ps[:])

    # M_z staging tiles (pre-zeroed; only diag blocks ever written)
    mz_all = {}
    for b in range(B):
        for p in range(NPAIR):
            mz = consts.tile([128, 128], F16, tag=f"mz{b}_{p}", name=f"mz{b}_{p}")
            nc.gpsimd.memset(mz[:], 0.0)
            mz_all[(b, p)] = mz

    # dots staging: [128, B * NPAIR * 64]
    dcols = B * NPAIR * 64
    dots_l = consts.tile([128, dcols], F32, tag="dots_l")
    dots_a = consts.tile([128, dcols], F32, tag="dots_a")

    xT_all = {}   # (b, cs) -> [128, n_chunk] f16
    kraw_all = {}  # (b, nt) -> [128, 1024] f16 (k | v)
    kt_all = {}
    cc_out_all = {}
    a_all = {}

    # ---------------- phase 1a ----------------
    x_tiles = {}  # b -> list of f16 [128, CIN] tiles

    def load_x(b):
        x_ts = []
        for nt in range(NT):
            x_t = x_pool.tile([128, CIN], F16, tag="x", name="x_t")
            nc.gpsimd.dma_start(x_t[:], x_d[b, nt * 128:(nt + 1) * 128, :])
            x_ts.append(x_t)
        x_tiles[b] = x_ts

    def phase1a(b):
        if b + 1 < B:
            load_x(b + 1)  # prefetch next batch on the gpsimd queue
        x_ts = x_tiles[b]
        for cs in range(2):
            xT = xT_pool.tile([128, n_chunk], F16, tag=f"xT{b}_{cs}",
                              name=f"xT{b}_{cs}")
            xT_all[(b, cs)] = xT
            for ntp in range(NT // 2):
                ps = xpose_ps.tile([128, 256], F16, tag="xp", name="ps")
                nc.tensor.transpose(
                    ps[:, 0:128], x_ts[2 * ntp][:, cs * 128:(cs + 1) * 128],
                    ident[:])
                nc.tensor.transpose(
                    ps[:, 128:256], x_ts[2 * ntp + 1][:, cs * 128:(cs + 1) * 128],
                    ident[:])
                nc.scalar.copy(xT[:, ntp * 256:(ntp + 1) * 256], ps[:])

        # S1 = sum_d k|v (from the PE mean-matmul), S2 = sum_d (k|v)^2
        s1 = stats_pool.tile([128, NT, 16], F32, tag="s1", name="s1")
        NT2 = NT // 2
        s2f = stats_pool.tile([128, NT, 16], F32, tag="s2f", name="s2f")
        kvraw_w = {}

        def stats(nt):
            w, i = divmod(nt, NT2)
            if i == 0:
                kvraw_w[w] = kv_sb.tile([128, NT2, 1024], F16, tag=f"kvraw{w}",
                                        bufs=2, name=f"kvraw{w}")
            kvr = kvraw_w[w]
            kraw_all[(b, nt)] = kvr[:, i, :]
            kps = kv_ps.tile([128, 512], F32, tag="kv", name="kps")
            vps = kv_ps.tile([128, 512], F32, tag="kv", name="vps")
            mps = sm_ps.tile([128, 256], F32, tag="sm", name="mps")
            for cs in range(2):
                xT_sl = xT_all[(b, cs)][:, nt * 128:(nt + 1) * 128]
                nc.tensor.matmul(kps[:], xT_sl, wkv_t[cs][:, :INNER],
                                 start=(cs == 0), stop=(cs == 1))
                nc.tensor.matmul(vps[:], xT_sl, wkv_t[cs][:, INNER:],
                                 start=(cs == 0), stop=(cs == 1))
                nc.tensor.matmul(mps[:, 0:16], xT_sl, m16_t[cs][:],
                                 start=(cs == 0), stop=(cs == 1))
            nc.scalar.copy(kvr[:, i, 0:512], kps[:])
            nc.scalar.copy(kvr[:, i, 512:1024], vps[:])
            nc.scalar.copy(s1[:, nt, :], mps[:, 0:16])

        def wave_stats(w):
            # one big square + a flat-2x fold chain (the segmented
            # tensor_reduce runs 1x; flat tensor_tensor adds run 2x)
            kvr = kvraw_w[w]
            sqw = kv_sb.tile([128, NT2 * 1024], F16, tag="sq", bufs=2, name="sq")
            nc.vector.tensor_tensor(sqw[:], kvr[:].rearrange("p t f -> p (t f)"),
                                    kvr[:].rearrange("p t f -> p (t f)"),
                                    op=mybir.AluOpType.mult)
            G = NT2 * 16  # segments of 64
            cur = sqw
            width = 64
            while width > 1:
                half = width // 2
                cv = cur.rearrange("p (g d) -> p g d", d=width)
                if half == 1:
                    nxt = None
                    with nc.allow_low_precision(reason="fold"):
                        nc.vector.tensor_tensor(
                            s2f[:, w * NT2:(w + 1) * NT2, :].rearrange(
                                "p t g -> p (t g)"),
                            cv[:, :, 0].rearrange("p g -> p (g)"),
                            cv[:, :, 1].rearrange("p g -> p (g)"),
                            op=mybir.AluOpType.add)
                else:
                    nxt = kv_sb.tile([128, G * half], F16, tag=f"fold{half}",
                                     bufs=2, name=f"fold{half}")
                    nc.vector.tensor_tensor(
                        nxt.rearrange("p (g d) -> p g d", d=half),
                        cv[:, :, 0:half], cv[:, :, half:width],
                        op=mybir.AluOpType.add)
                cur = nxt
                width = half

        def combine_apply(w):
            sl = slice(w * NT2, (w + 1) * NT2)
            # var*4096 = 64*S2 - S1^2 ;  a = 4096*rsqrt(prodk*prodv)
            t1 = small_pool.tile([128, NT2, 16], F32, tag=f"t1{w}")
            nc.vector.tensor_tensor(t1[:], s1[:, sl, :], s1[:, sl, :],
                                    op=mybir.AluOpType.mult)
            t2 = small_pool.tile([128, NT2, 16], F32, tag=f"t2{w}")
            nc.vector.scalar_tensor_tensor(
                t2[:], s2f[:, sl, :], 64.0, t1[:],
                op0=mybir.AluOpType.mult, op1=mybir.AluOpType.subtract)
            nc.vector.tensor_scalar_add(t2[:], t2[:], 4096.0 * EPS)
            prod = small_pool.tile([128, NT2, 8], F32, tag=f"prod{w}")
            nc.vector.tensor_tensor(prod[:], t2[:, :, 0:8], t2[:, :, 8:16],
                                    op=mybir.AluOpType.mult)
            nc.scalar.activation(prod[:], prod[:],
                                 mybir.ActivationFunctionType.Sqrt)
            recp = small_pool.tile([128, NT2, 8], F32, tag=f"recp{w}")
            nc.vector.reciprocal(recp[:], prod[:])
            a_sc = small_pool.tile([128, NT2, 8], F16, tag=f"asc{w}")
            nc.vector.tensor_scalar_mul(a_sc[:], recp[:], 4096.0)
            aexp = small_pool.tile([128, NT2, 512], F16, tag=f"aexp{w}")
            nc.vector.tensor_copy(
                aexp.rearrange("p t (h d) -> p t h d", d=D),
                a_sc.broadcast_to([128, NT2, 8, D]))
            for i in range(NT2):
                nt = w * NT2 + i
                kt = kv_sb.tile([128, 512], F16, tag="kt", name="kt")
                kt_all[(b, nt)] = kt
                nc.vector.tensor_tensor(kt[:], kvraw_w[w][:, i, 0:512],
                                        aexp[:, i, :], op=mybir.AluOpType.mult)

        for nt in range(NT2):
            stats(nt)
        wave_stats(0)
        combine_apply(0)
        for nt in range(NT2, NT):
            stats(nt)
        wave_stats(1)
        combine_apply(1)

    # ---------------- dots + AllReduce ----------------
    # ---------------- dots + AllReduce ----------------
    def dots_ar(b):
        acc = dots_ps.tile([128, 512], F32, tag="dots", name="acc")
        for p in range(NPAIR):
            for nt in range(NT):
                nc.tensor.matmul(
                    acc[:, p * 128:(p + 1) * 128],
                    kt_all[(b, nt)][:, p * 128:(p + 1) * 128],
                    kraw_all[(b, nt)][:, 512 + p * 128:512 + (p + 1) * 128],
                    start=(nt == 0), stop=(nt == NT - 1))
        for p in range(NPAIR):
            col = (b * NPAIR + p) * 64
            nc.scalar.copy(dots_l[0:64, col:col + 64],
                           acc[0:64, p * 128:p * 128 + 64])
            nc.scalar.copy(dots_l[64:128, col:col + 64],
                           acc[64:128, p * 128 + 64:(p + 1) * 128])
        bcols = NPAIR * 64
        cc_in = dram.tile([128, bcols], F32, tag="cc_in", bufs=B, name=f"cc_in{b}")
        cc_out = dram.tile([128, bcols], F32, tag="cc_out", bufs=B, name=f"cc_out{b}")
        bsl = slice(b * bcols, (b + 1) * bcols)
        nc.gpsimd.dma_start(cc_in[:], dots_l[:, bsl])
        nc.gpsimd.collective_compute(
            "AllReduce", mybir.AluOpType.add,
            replica_groups=[list(range(ncores))],
            ins=[cc_in.opt()], outs=[cc_out.opt()])
        cc_out_all[b] = cc_out

    # ---------------- phase 2: fold dots into weights, out proj ----------------
    def phase2(b):
        bcols = NPAIR * 64
        bsl = slice(b * bcols, (b + 1) * bcols)
        nc.sync.dma_start(dots_a[:, bsl], cc_out_all[b][:])
        wt_sb = []
        for p in range(NPAIR):
            col = (b * NPAIR + p) * 64
            mz = mz_all[(b, p)]
            nc.scalar.copy(mz[0:64, 0:64], dots_a[0:64, col:col + 64])
            nc.scalar.copy(mz[64:128, 64:128], dots_a[64:128, col:col + 64])
            ps = sm_ps.tile([128, 256], F32, tag="sm", name="ps_r")
            nc.tensor.matmul(ps[:], mz[:], wq_p[p][:], start=True, stop=True)
            r_sb = p2_pool.tile([128, 256], F16, tag="r_sb", name="r_sb")
            nc.scalar.copy(r_sb[:], ps[:])
            ps2 = sm_ps.tile([128, 256], F32, tag="sm", name="ps_w")
            nc.tensor.matmul(ps2[:], cmat[:], r_sb[:], start=True, stop=True)
            wt = p2_pool.tile([128, 256], F16, tag="wt", bufs=2 * NPAIR, name="wt")
            nc.scalar.copy(wt[:], ps2[:])
            wt_sb.append(wt)
        gt_sb = []
        for cs in range(2):
            psg = sm_ps.tile([128, 256], F32, tag="sm", name="ps_g")
            for j in range(NPAIR):
                nc.tensor.matmul(psg[:], wt_sb[j][:, cs * 128:(cs + 1) * 128],
                                 wo_t[j][:], start=(j == 0), stop=(j == NPAIR - 1))
            gt = p2_pool.tile([128, 256], F16, tag="gt", name="gt")
            nc.scalar.copy(gt[:], psg[:])
            gt_sb.append(gt)
        for nt in range(NT):
            pso = sm_ps.tile([128, 256], F32, tag="sm", name="ps_o")
            for cs in range(2):
                nc.tensor.matmul(pso[:],
                                 xT_all[(b, cs)][:, nt * 128:(nt + 1) * 128],
                                 gt_sb[cs][:], start=(cs == 0), stop=(cs == 1))
            osb = out_pool.tile([128, COUT], F32, tag="osb", name="osb")
            nc.vector.tensor_tensor(osb[:], pso[:], bias_bc[:],
                                    op=mybir.AluOpType.add)
            nc.sync.dma_start(out_d[b, nt * 128:(nt + 1) * 128, :], osb[:])

    # schedule: keep PE fed; dots(b) emitted after phase1a(b+1) so the
    # normalization (vector) of batch b overlaps the projections of b+1;
    # phase2(b) emitted ~2 phases after its AllReduce was issued.
    load_x(0)
    phase1a(0)
    dots_ar(0)
    phase1a(1)
    dots_ar(1)
    phase1a(2)
    dots_ar(2)
    phase2(0)
    phase1a(3)
    dots_ar(3)
    phase2(1)
    phase2(2)
    phase2(3)


_NC_CACHE = {}


def _get_nc(n_chunk, n_full, ncores):
    key = (n_chunk, n_full, ncores)
    if key not in _NC_CACHE:
        _NC_CACHE[key] = _build(n_chunk, n_full, ncores)
    return _NC_CACHE[key]


def _make_in_maps(u_x, Wq, Wk, Wv, Wo, bo, ncores):
    n = u_x.shape[1]
    n_chunk = n // ncores
    wq = np.ascontiguousarray(np.asarray(Wq, np.float32))
    wk = np.ascontiguousarray(np.asarray(Wk, np.float32))
    wv = np.ascontiguousarray(np.asarray(Wv, np.float32))
    wo = np.ascontiguousarray(np.asarray(Wo, np.float32))
    bo2 = np.ascontiguousarray(np.asarray(bo, np.float32).reshape(1, -1))
    u_x = np.asarray(u_x, np.float32)
    maps = []
    for c in range(ncores):
        maps.append({
            "x": np.ascontiguousarray(u_x[:, c * n_chunk:(c + 1) * n_chunk, :]),
            "wq": wq, "wk": wk, "wv": wv, "wo": wo, "bo": bo2,
        })
    return maps, n_chunk


def _install_ntff_hook():
    """Provide antenv.axon_hooks (missing in this image) so trace=True works."""
    import types
    try:
        from antenv.axon_hooks import get_axon_ntff_profile_hook  # noqa: F401
        return  # real module present
    except ImportError:
        pass
    try:
        import antenv
        mod = types.ModuleType("antenv.axon_hooks")
        _state = {"hook": None}
        mod.set_axon_ntff_profile_hook = lambda h: _state.__setitem__("hook", h)
        mod.get_axon_ntff_profile_hook = lambda: _state["hook"]
        sys.modules["antenv.axon_hooks"] = mod
        antenv.axon_hooks = mod
        boot_dir = "/root/.axon_site/trn_agent_boot"
        if boot_dir not in sys.path and os.path.isdir(boot_dir):
            sys.path.insert(0, boot_dir)
        import trn_boot
        so_path = "/opt/axon/libaxon_pjrt.so"
        if os.path.exists(so_path):
            hook = trn_boot._ntff_profile_via_ctypes(so_path)
            if hook is not None:
                mod.set_axon_ntff_profile_hook(hook)
    except Exception as e:  # tracing is best-effort; never break the run path
        print(f"ntff hook install failed: {e}", file=sys.stderr)


def run(u_x, Wq, Wk, Wv, Wo, bo, n_full=None, ncores=NCORES, trace=False,
        tmpdir=None):
    if trace:
        _install_ntff_hook()
    n = u_x.shape[1]
    if n_full is None:
        n_full = n
    in_maps, n_chunk = _make_in_maps(u_x, Wq, Wk, Wv, Wo, bo, ncores)
    nc = _get_nc(n_chunk, n_full, ncores)
    res = run_bass_kernel_spmd(nc, in_maps, list(range(ncores)), trace=trace,
                               tmpdir=tmpdir)
    outs = [np.asarray(res.results[c]["out"]) for c in range(ncores)]
    full = np.concatenate(outs, axis=1).astype(np.float32)
    return full, res


def kernel(u_x, pos_x=None, Wq=None, Wk=None, Wv=None, Wo=None, bo=None):
    full, _ = run(np.asarray(u_x, np.float32), Wq, Wk, Wv, Wo, bo)
    return full
